# revision 1
# baseline (speedup 1.0000x reference)
"""Trainium2 Bass kernel for nn_ModelName_86242943303934 (gnn_message_passing).

Self-contained: takes FULL inputs, shards across 8 NeuronCores internally,
runs one SPMD Bass/Tile program, gathers the full [2048, 1] output.

Decomposition (validated against the jax reference in a numpy prototype):
  - hypergraph propagation (2 layers) for H_ug / H_ug_affect / H_gg with
    row-sharded H (fp8 0/1 entries), bf16 activations, fp32 PSUM accumulate,
    bf16 AllReduce of the per-core partial s = H^T x.
  - degrees de/dv precomputed on host (0.05% of FLOPs), folded in as
    reciprocal rows.
  - choose_emb via host-gathered, dv-scaled H_gg[groupid] rows (one matmul).
  - ragged member attention: device dma_gather of packed [user | user@W1u]
    rows from an AllGathered table; per-member MLP via vector/scalar engines;
    segment softmax-sum via host-built one-hot S matrices (matmul).
"""
import sys
sys.path.insert(0, '/opt/trn_rl_repo')

import numpy as np
import ml_dtypes

import concourse.bass as bass
import concourse.mybir as mybir
import concourse.tile as tile
from concourse import bacc
from concourse.bass_utils import run_bass_kernel_spmd
from concourse.masks import make_identity

bf16 = ml_dtypes.bfloat16
f8 = ml_dtypes.float8_e4m3fn
FP32 = mybir.dt.float32
BF16 = mybir.dt.bfloat16
F8 = mybir.dt.float8e4
I16 = mybir.dt.int16

NC = 8
U, G, D, B = 30000, 4096, 128, 2048
UC = U // NC            # 3750 local users
KU = 30                 # user chunks of 128 (padded)
UCP = KU * 128          # 3840
GS = 8                  # g-subtiles of 512 in pass A
USUB = 480              # pass-B u-subtile width (8 * 480 = 3840)
NUS = 8
GGR = G // NC           # 512 local H_gg rows
KG = 4                  # gg chunks of 128
BC = B // NC            # 256 batch rows per core
NGC = 32                # g chunks of 128

AF = mybir.ActivationFunctionType


def _wrap_idx(idx, n):
    cols = (n + 15) // 16
    w = np.zeros((16, cols), np.int16)
    for i in range(n):
        w[i % 16, i // 16] = idx[i]
    return np.tile(w, (8, 1))


def _prep(inputs):
    inp = {k: np.asarray(v) for k, v in inputs.items()}
    H = {'a': inp['H_ug'].astype(np.float32),
         'b': inp['H_ug_affect'].astype(np.float32)}
    Hg = inp['H_gg'].astype(np.float32)
    user_emb = inp['user_emb'].astype(np.float32)
    group_emb = inp['group_emb'].astype(np.float32)
    item_emb = inp['item_emb'].astype(np.float32)
    groupid = inp['groupid'].astype(np.int64)
    itemid = inp['itemid'].astype(np.int64)
    mids = inp['member_user_ids'].astype(np.int64)
    bseg = inp['batch_seg'].astype(np.int64)

    att_w1 = inp['att_w1'].astype(np.float32)
    att_b1 = inp['att_b1'].astype(np.float32)
    att_w2 = inp['att_w2'].astype(np.float32)
    pw1 = inp['pred_w1'].astype(np.float32)
    pb1 = inp['pred_b1'].astype(np.float32)
    pw2 = inp['pred_w2'].astype(np.float32)

    deg = {}
    for m, Hm in (('a', H['a']), ('b', H['b']), ('g', Hg)):
        deg[m] = (Hm.sum(1) + 1e-5, Hm.sum(0) + 1e-5)

    counts = np.bincount(bseg, minlength=B)
    starts = np.concatenate([[0], np.cumsum(counts)])
    mc = [int(starts[(c + 1) * BC] - starts[c * BC]) for c in range(NC)]
    MPAD = int(-(-max(mc) // 128) * 128)
    NJ = MPAD // 128

    item_b = item_emb[itemid]                      # [B, D] host gather of inputs

    in_maps = []
    for c in range(NC):
        m = {}
        for k in ('a', 'b'):
            rows = slice(c * UC, (c + 1) * UC)
            Hp = np.zeros((UCP, G), np.float32)
            Hp[:UC] = H[k][rows]
            m[f'hu_{k}'] = Hp.astype(f8)
            HT = Hp.T.reshape(NGC, 128, NUS, USUB).transpose(2, 1, 0, 3)
            m[f'hut_{k}'] = np.ascontiguousarray(
                HT.reshape(NUS, 128, NGC * USUB)).astype(f8)
            dv, de = deg[k]
            dvp = np.zeros((1, UCP), np.float32)
            dvp[0, :UC] = 1.0 / dv[rows]
            m[f'dvr_{k}'] = np.tile(dvp, (128, 1))
            m[f'der_{k}'] = np.tile((1.0 / de)[None, :], (128, 1)).astype(bf16)
        x0 = np.zeros((UCP, D), np.float32)
        x0[:UC] = user_emb[c * UC:(c + 1) * UC]
        m['x0u'] = np.ascontiguousarray(
            x0.reshape(KU, 128, D).transpose(1, 0, 2)).astype(bf16)

        rows = slice(c * GGR, (c + 1) * GGR)
        m['hg'] = Hg[rows].astype(f8)
        HTg = Hg[rows].T.reshape(NGC, 128, GGR).transpose(1, 0, 2)
        m['hgt'] = np.ascontiguousarray(HTg.reshape(128, NGC * GGR)).astype(f8)
        dv, de = deg['g']
        m['dvr_g'] = np.tile((1.0 / dv[rows])[None, :], (128, 1)).astype(np.float32)
        m['der_g'] = np.tile((1.0 / de)[None, :], (128, 1)).astype(bf16)
        m['xg0'] = np.ascontiguousarray(
            group_emb[rows].reshape(KG, 128, D).transpose(1, 0, 2)).astype(bf16)

        bid = slice(c * BC, (c + 1) * BC)
        gid = groupid[bid]
        Hgr = Hg[gid] / deg['g'][0][gid][:, None]          # [BC, G]
        HgrT = Hgr.T.reshape(NGC, 128, 2, 128).transpose(1, 0, 2, 3)
        m['hgrt'] = np.ascontiguousarray(
            HgrT.reshape(128, NGC * 2 * 128)).astype(bf16)

        m['item_bt'] = np.ascontiguousarray(item_b[bid].T).astype(bf16)
        mlo, mhi = int(starts[c * BC]), int(starts[(c + 1) * BC])
        mid_c = mids[mlo:mhi]
        seg_c = (bseg[mlo:mhi] - c * BC).astype(np.int64)
        Mc = len(mid_c)
        gi = (mid_c // UC) * UCP + (mid_c % UC)
        gi = np.concatenate([gi, np.zeros(MPAD - Mc, np.int64)])
        m['gidx'] = _wrap_idx(gi.astype(np.int16), MPAD)
        S_bm = np.zeros((NJ, BC, 128), np.float32)
        S_mb = np.zeros((NJ, 128, BC), np.float32)
        jj, pp = np.arange(Mc) // 128, np.arange(Mc) % 128
        S_bm[jj, seg_c, pp] = 1.0
        S_mb[jj, pp, seg_c] = 1.0
        sbm = S_bm.reshape(NJ, 2, 128, 128).transpose(2, 0, 1, 3)
        smb = S_mb.reshape(NJ, 128, 2, 128).transpose(1, 0, 2, 3)
        m['s_bm'] = np.ascontiguousarray(sbm.reshape(128, NJ * 2 * 128)).astype(bf16)
        m['s_mb'] = np.ascontiguousarray(smb.reshape(128, NJ * 2 * 128)).astype(bf16)

        m['w1u'] = att_w1[:D].astype(bf16)
        m['w1i'] = att_w1[D:].astype(bf16)
        m['pw1'] = np.ascontiguousarray(
            pw1.reshape(3, 128, 8).transpose(1, 0, 2).reshape(128, 24)).astype(bf16)
        crow = np.zeros((1, 48), np.float32)
        crow[0, 0:16] = att_b1
        crow[0, 16:32] = att_w2[:, 0]
        crow[0, 32:40] = pb1
        crow[0, 40:48] = pw2[:, 0]
        m['crow'] = np.tile(crow, (128, 1))
        in_maps.append(m)

    meta = dict(MPAD=MPAD, NJ=NJ,
                att_b2=float(inp['att_b2'][0]), pred_b2=float(inp['pred_b2'][0]))
    return in_maps, meta


def _build(meta):
    NJ, MPAD = meta['NJ'], meta['MPAD']
    att_b2, pred_b2 = meta['att_b2'], meta['pred_b2']

    nc = bacc.Bacc("TRN2", target_bir_lowering=False)

    def din(name, shape, dt):
        return nc.dram_tensor(name, list(shape), dt, kind="ExternalInput")

    hu = {k: din(f'hu_{k}', (UCP, G), F8) for k in 'ab'}
    hut = {k: din(f'hut_{k}', (NUS, 128, NGC * USUB), F8) for k in 'ab'}
    dvr = {k: din(f'dvr_{k}', (128, UCP), FP32) for k in 'ab'}
    der = {k: din(f'der_{k}', (128, G), BF16) for k in 'ab'}
    x0u = din('x0u', (128, KU, D), BF16)
    hg = din('hg', (GGR, G), F8)
    hgt = din('hgt', (128, NGC * GGR), F8)
    dvr['g'] = din('dvr_g', (128, GGR), FP32)
    der['g'] = din('der_g', (128, G), BF16)
    xg0 = din('xg0', (128, KG, D), BF16)
    hgrt = din('hgrt', (128, NGC * 2 * 128), BF16)
    item_bt = din('item_bt', (128, 2 * 128), BF16)
    gidx = din('gidx', (128, MPAD // 16), I16)
    s_bm = din('s_bm', (128, NJ * 2 * 128), BF16)
    s_mb = din('s_mb', (128, NJ * 2 * 128), BF16)
    w1u = din('w1u', (D, 16), BF16)
    w1i = din('w1i', (D, 16), BF16)
    pw1 = din('pw1', (128, 24), BF16)
    crow = din('crow', (128, 48), FP32)
    out = nc.dram_tensor('out', [BC, 1], FP32, kind="ExternalOutput")

    RG = [list(range(NC))]

    with tile.TileContext(nc) as tc:
        with (
            tc.tile_pool(name="pers", bufs=1) as pers,
            tc.tile_pool(name="ps", bufs=1, space="PSUM") as ps,
            tc.tile_pool(name="dram", bufs=1, space="DRAM") as dr,
        ):
            # ---------------- persistent small tiles ----------------
            w1u_sb = pers.tile([D, 16], BF16, name="w1u_sb")
            nc.sync.dma_start(w1u_sb[:], w1u[:])
            w1i_sb = pers.tile([D, 16], BF16, name="w1i_sb")
            nc.sync.dma_start(w1i_sb[:], w1i[:])
            pw1_sb = pers.tile([128, 3, 8], BF16, name="pw1_sb")
            nc.sync.dma_start(pw1_sb[:], pw1[:].rearrange("p (k o) -> p k o", k=3))
            crow_sb = pers.tile([128, 48], FP32, name="crow_sb")
            nc.sync.dma_start(crow_sb[:], crow[:])
            crow16 = pers.tile([128, 48], BF16, name="crow16")
            nc.vector.tensor_copy(crow16[:], crow_sb[:])
            ibt_sb = pers.tile([128, 256], BF16, name="ibt_sb")
            nc.sync.dma_start(ibt_sb[:], item_bt[:])
            ident = pers.tile([128, 128], FP32, name="ident")
            make_identity(nc, ident[:])
            choose_sb = pers.tile([128, 2, 128], FP32, name="choose_sb")

            # DRAM internals
            ar_in = {(k, it): dr.tile([128, G], BF16, name=f"arin_{k}{it}",
                                      tag=f"arin{k}{it}")
                     for k in 'abg' for it in range(2)}
            ar_out = {(k, it): dr.tile([128, G], BF16, name=f"arout_{k}{it}",
                                       tag=f"arout{k}{it}", addr_space="Shared")
                      for k in 'abg' for it in range(2)}
            snT_dram = {k: dr.tile([128, G], BF16, name=f"snT_{k}", tag=f"snT{k}")
                        for k in 'abg'}
            xT_dram = {k: dr.tile([128, UCP], BF16, name=f"xT_{k}", tag=f"xT{k}")
                       for k in 'ab'}
            xT_dram['g'] = dr.tile([128, GGR], BF16, name="xT_g", tag="xTg")
            userT_dram = dr.tile([128, UCP], BF16, name="userT_dram")
            upT_dram = dr.tile([16, UCP], BF16, name="upT_dram")
            table_loc = dr.tile([UCP, 256], BF16, name="table_loc")
            table_full = dr.tile([NC * UCP, 256], BF16, name="table_full",
                                 addr_space="Shared")

            KCH = {'a': KU, 'b': KU, 'g': KG}

            # ================= propagation phase =================
            with (
                tc.tile_pool(name="hk_pool", bufs=3) as hkp,
                tc.tile_pool(name="panel_pool", bufs=2) as plp,
                tc.tile_pool(name="prop", bufs=1) as prop,
            ):
                HU = {'a': hu['a'], 'b': hu['b'], 'g': hg}
                x_sb = {}
                x_sb['a'] = prop.tile([128, KU, D], BF16, name="xa_sb", tag="xa")
                nc.sync.dma_start(x_sb['a'][:], x0u[:])
                x_sb['b'] = prop.tile([128, KU, D], BF16, name="xb_sb", tag="xb")
                nc.sync.dma_start(x_sb['b'][:], x0u[:])
                x_sb['g'] = prop.tile([128, KG, D], BF16, name="xg_sb", tag="xg")
                nc.sync.dma_start(x_sb['g'][:], xg0[:])
                sn_tiles = {k: prop.tile([128, NGC, D], BF16, name=f"sn_{k}",
                                         tag=f"sn{k}") for k in 'abg'}
                x1T_bf = {
                    'a': prop.tile([128, UCP], BF16, name="x1Ta", tag="x1Ta"),
                    'b': prop.tile([128, UCP], BF16, name="x1Tb", tag="x1Tb"),
                }

                def pass_a(mat, it):
                    kch = KCH[mat]
                    psum = [ps.tile([128, 512], FP32, name=f"pa{gs}", tag=f"pa{gs}")
                            for gs in range(GS)]
                    for k in range(kch):
                        hk = hkp.tile([128, G], F8, name="hk", tag="hk")
                        nc.sync.dma_start(hk[:], HU[mat][k * 128:(k + 1) * 128, :])
                        for gs in range(GS):
                            nc.tensor.matmul(
                                psum[gs][:], lhsT=x_sb[mat][:, k, :],
                                rhs=hk[:, gs * 512:(gs + 1) * 512],
                                start=(k == 0), stop=(k == kch - 1))
                    stage = prop.tile([128, G], BF16, name="stage", tag="arstage")
                    for gs in range(GS):
                        nc.vector.tensor_copy(
                            stage[:, gs * 512:(gs + 1) * 512], psum[gs][:])
                    nc.sync.dma_start(ar_in[(mat, it)][:], stage[:])
                    nc.gpsimd.collective_compute(
                        "AllReduce", mybir.AluOpType.add,
                        ins=[ar_in[(mat, it)].opt()], outs=[ar_out[(mat, it)].opt()],
                        replica_groups=RG)

                def norm_transpose(mat, it):
                    sAR = prop.tile([128, G], BF16, name="sAR", tag="sAR")
                    nc.sync.dma_start(sAR[:], ar_out[(mat, it)][:])
                    derr = prop.tile([128, G], BF16, name="derr", tag="derr")
                    nc.sync.dma_start(derr[:], der[mat][:])
                    snT = prop.tile([128, G], BF16, name="snT", tag="snTsb")
                    nc.vector.tensor_tensor(
                        out=snT[:], in0=sAR[:],
                        in1=derr[:],
                        op=mybir.AluOpType.mult)
                    nc.sync.dma_start(snT_dram[mat][:], snT[:])
                    for gc in range(NGC):
                        nc.sync.dma_start(
                            sn_tiles[mat][:, gc, :],
                            snT_dram[mat][:, gc * 128:(gc + 1) * 128],
                            transpose=True)

                def pass_b(mat, last):
                    if mat == 'g':
                        nsub, usub, width = 1, GGR, GGR
                        x1t = prop.tile([128, GGR], BF16, name="x1Tg", tag="x1Tg")
                    else:
                        nsub, usub, width = NUS, USUB, UCP
                        x1t = x1T_bf[mat]
                    for us in range(nsub):
                        panel = plp.tile([128, NGC * usub], F8, name="panel",
                                         tag="panel")
                        src = hgt[:] if mat == 'g' else hut[mat][us]
                        nc.sync.dma_start(panel[:], src)
                        pb = ps.tile([128, usub], FP32, name="pb", tag="pa0")
                        for gc in range(NGC):
                            nc.tensor.matmul(
                                pb[:], lhsT=sn_tiles[mat][:, gc, :],
                                rhs=panel[:, gc * usub:(gc + 1) * usub],
                                start=(gc == 0), stop=(gc == NGC - 1))
                        dvsl = prop.tile([128, 512], FP32, name="dvsl", tag="dvsl")
                        nc.sync.dma_start(dvsl[:, :usub],
                                          dvr[mat][:, us * usub:(us + 1) * usub])
                        nc.vector.tensor_tensor(
                            out=x1t[:, us * usub:(us + 1) * usub], in0=pb[:],
                            in1=dvsl[:, :usub],
                            op=mybir.AluOpType.mult)
                    if not last:
                        nc.sync.dma_start(xT_dram[mat][:, :width], x1t[:, :width])
                        for k in range(KCH[mat]):
                            nc.sync.dma_start(
                                x_sb[mat][:, k, :],
                                xT_dram[mat][:, k * 128:(k + 1) * 128],
                                transpose=True)

                for it in range(2):
                    for mat in 'abg':
                        pass_a(mat, it)
                    for mat in 'abg':
                        norm_transpose(mat, it)
                        if it == 0:
                            pass_b(mat, last=False)
                        elif mat != 'g':
                            pass_b(mat, last=True)

                # ---------- choose ----------
                hgrt_sb = prop.tile([128, NGC, 2, 128], BF16, name="hgrt_sb")
                nc.sync.dma_start(
                    hgrt_sb[:],
                    hgrt[:].rearrange("p (g h b) -> p g h b", g=NGC, h=2))
                ps_ch = [ps.tile([128, 128], FP32, name=f"ch{h}", tag=f"pa{1 + h}")
                         for h in range(2)]
                for gc in range(NGC):
                    for h in range(2):
                        nc.tensor.matmul(
                            ps_ch[h][:], lhsT=hgrt_sb[:, gc, h, :],
                            rhs=sn_tiles['g'][:, gc, :],
                            start=(gc == 0), stop=(gc == NGC - 1))
                for h in range(2):
                    nc.vector.tensor_copy(choose_sb[:, h, :], ps_ch[h][:])

                # ---------- user combine + table build ----------
                userT = prop.tile([128, UCP], BF16, name="userT")
                nc.vector.tensor_add(userT[:], x1T_bf['a'][:], x1T_bf['b'][:])
                nc.scalar.activation(userT[:], userT[:], AF.Copy, scale=0.5)
                upT = prop.tile([16, UCP], BF16, name="upT")
                for us in range(NUS):
                    pu = ps.tile([16, USUB], FP32, name="pu", tag="pa3")
                    nc.tensor.matmul(pu[:], lhsT=w1u_sb[:],
                                     rhs=userT[:, us * USUB:(us + 1) * USUB],
                                     start=True, stop=True)
                    nc.vector.tensor_copy(upT[:, us * USUB:(us + 1) * USUB], pu[:])
                nc.sync.dma_start(userT_dram[:], userT[:])
                nc.sync.dma_start(upT_dram[:], upT[:])
                tbl = prop.tile([128, KU, 256], BF16, name="tbl")
                nc.vector.memset(tbl[:], 0.0)
                for k in range(KU):
                    nc.sync.dma_start(tbl[:, k, 0:128],
                                      userT_dram[:, k * 128:(k + 1) * 128],
                                      transpose=True)
                    nc.sync.dma_start(tbl[:, k, 128:144],
                                      upT_dram[:, k * 128:(k + 1) * 128],
                                      transpose=True)
                nc.sync.dma_start(
                    table_loc[:].rearrange("(k p) e -> p k e", p=128), tbl[:])
                nc.gpsimd.collective_compute(
                    "AllGather", mybir.AluOpType.bypass,
                    ins=[table_loc.opt()], outs=[table_full.opt()],
                    replica_groups=RG)

            # ================= tail =================
            with tc.tile_pool(name="wtp", bufs=1) as wtp:
                wt = wtp.tile([128, NJ, 132], BF16, name="wt")
                att_bf = wtp.tile([128, NJ], BF16, name="att_bf")

                with tc.tile_pool(name="tailA", bufs=1) as ta:
                    idx_sb = ta.tile([128, MPAD // 16], I16, name="idx_sb")
                    nc.sync.dma_start(idx_sb[:], gidx[:])
                    gath = ta.tile([128, NJ, 256], BF16, name="gath")
                    nc.gpsimd.dma_gather(
                        out_ap=gath[:], in_ap=table_full[:], idxs_ap=idx_sb[:],
                        num_idxs=MPAD, num_idxs_reg=MPAD, elem_size=256,
                        single_packet=False)

                    sbm_sb = ta.tile([128, NJ, 2, 128], BF16, name="sbm_sb")
                    nc.sync.dma_start(
                        sbm_sb[:],
                        s_bm[:].rearrange("p (j h m) -> p j h m", j=NJ, h=2))

                    iproj = ta.tile([128, 2, 16], BF16, name="iproj")
                    for h in range(2):
                        pi = ps.tile([128, 16], FP32, name="pi", tag="pa4")
                        nc.tensor.matmul(pi[:],
                                         lhsT=ibt_sb[:, h * 128:(h + 1) * 128],
                                         rhs=w1i_sb[:], start=True, stop=True)
                        nc.vector.tensor_copy(iproj[:, h, :], pi[:])
                    nc.vector.tensor_tensor(
                        out=iproj[:], in0=iproj[:],
                        in1=crow16[:, 0:16].unsqueeze(1)
                            .to_broadcast([128, 2, 16]),
                        op=mybir.AluOpType.add)

                    ip_all = ta.tile([128, NJ, 16], BF16, name="ip_all")
                    for j in range(NJ):
                        pj = ps.tile([128, 16], FP32, name="pj", tag="pa5")
                        for h in range(2):
                            nc.tensor.matmul(pj[:], lhsT=sbm_sb[:, j, h, :],
                                             rhs=iproj[:, h, :],
                                             start=(h == 0), stop=(h == 1))
                        nc.vector.tensor_copy(ip_all[:, j, :], pj[:])

                    h_all = ta.tile([128, NJ, 16], BF16, name="h_all")
                    nc.vector.tensor_add(h_all[:], gath[:, :, 128:144], ip_all[:])
                    nc.scalar.activation(h_all[:], h_all[:], AF.Relu)
                    hw = ta.tile([128, NJ, 16], FP32, name="hw")
                    nc.vector.tensor_tensor(
                        out=hw[:], in0=h_all[:],
                        in1=crow16[:, 16:32].unsqueeze(1)
                            .to_broadcast([128, NJ, 16]),
                        op=mybir.AluOpType.mult)
                    logit = ta.tile([128, NJ], FP32, name="logit")
                    nc.vector.reduce_sum(logit[:], hw[:], axis=mybir.AxisListType.X)
                    att = ta.tile([128, NJ], FP32, name="att")
                    nc.scalar.activation(att[:], logit[:], AF.Exp, bias=att_b2)
                    nc.vector.tensor_copy(att_bf[:], att[:])

                    nc.vector.tensor_tensor(
                        out=wt[:, :, 0:128], in0=gath[:, :, 0:128],
                        in1=att_bf[:].unsqueeze(2).to_broadcast([128, NJ, 128]),
                        op=mybir.AluOpType.mult)
                    nc.vector.tensor_copy(wt[:, :, 128:129], att_bf[:].unsqueeze(2))

                with tc.tile_pool(name="tailB", bufs=1) as tb:
                    smb_sb = tb.tile([128, NJ, 2, 128], BF16, name="smb_sb")
                    nc.sync.dma_start(
                        smb_sb[:],
                        s_mb[:].rearrange("p (j h b) -> p j h b", j=NJ, h=2))
                    ps_ag = [ps.tile([128, 129], FP32, name=f"ag{h}",
                                     tag=f"pa{6 + h}") for h in range(2)]
                    for j in range(NJ):
                        for h in range(2):
                            nc.tensor.matmul(ps_ag[h][:], lhsT=smb_sb[:, j, h, :],
                                             rhs=wt[:, j, 0:129],
                                             start=(j == 0), stop=(j == NJ - 1))

                    gT = tb.tile([128, 2, 128], BF16, name="gT")
                    for h in range(2):
                        den_r = tb.tile([128, 1], FP32, name="den_r", tag="den_r")
                        nc.vector.reciprocal(den_r[:], ps_ag[h][:, 128:129])
                        grp = tb.tile([128, 128], FP32, name="grp", tag="grp")
                        nc.vector.tensor_tensor(
                            out=grp[:], in0=ps_ag[h][:, 0:128],
                            in1=den_r[:].to_broadcast([128, 128]),
                            op=mybir.AluOpType.mult)
                        nc.vector.tensor_add(grp[:], grp[:], choose_sb[:, h, :])
                        pt = ps.tile([128, 128], FP32, name="pt", tag="pa4")
                        nc.tensor.transpose(pt[:], grp[:], ident[:])
                        nc.vector.tensor_copy(gT[:, h, :], pt[:])

                    giT = tb.tile([128, 2, 128], BF16, name="giT")
                    nc.vector.tensor_tensor(
                        out=giT[:], in0=gT[:],
                        in1=ibt_sb[:].rearrange("p (h b) -> p h b", h=2),
                        op=mybir.AluOpType.mult)

                    out_sb = tb.tile([128, 2], FP32, name="out_sb")
                    for h in range(2):
                        pp = ps.tile([128, 8], FP32, name="pp", tag="pa5")
                        ne = [giT[:, h, :], gT[:, h, :],
                              ibt_sb[:, h * 128:(h + 1) * 128]]
                        for kk in range(3):
                            nc.tensor.matmul(pp[:], lhsT=ne[kk],
                                             rhs=pw1_sb[:, kk, :],
                                             start=(kk == 0), stop=(kk == 2))
                        h2 = tb.tile([128, 8], FP32, name="h2", tag="h2")
                        nc.vector.tensor_tensor(
                            out=h2[:], in0=pp[:],
                            in1=crow_sb[:, 32:40],
                            op=mybir.AluOpType.add)
                        nc.scalar.activation(h2[:], h2[:], AF.Relu)
                        nc.vector.tensor_tensor(
                            out=h2[:], in0=h2[:],
                            in1=crow_sb[:, 40:48],
                            op=mybir.AluOpType.mult)
                        l2 = tb.tile([128, 1], FP32, name="l2", tag="l2")
                        nc.vector.reduce_sum(l2[:], h2[:],
                                             axis=mybir.AxisListType.X)
                        nc.scalar.activation(out_sb[:, h:h + 1], l2[:],
                                             AF.Sigmoid, bias=pred_b2)
                    nc.sync.dma_start(
                        out[:].rearrange("(h p) o -> p h o", p=128),
                        out_sb[:].unsqueeze(2))

    nc.finalize()
    return nc


def kernel(**inputs):
    in_maps, meta = _prep(inputs)
    nc = _build(meta)
    res = run_bass_kernel_spmd(nc, in_maps, list(range(NC)))
    outs = [res.results[c]['out'] for c in range(NC)]
    return np.concatenate(outs, axis=0).astype(np.float32)



# revision 5
# speedup vs baseline: 1.5422x; 1.5422x over previous
"""Trainium2 Bass kernel for nn_ModelName_86242943303934 (gnn_message_passing).

Self-contained: takes FULL inputs, shards across 8 NeuronCores internally,
runs one SPMD Bass/Tile program, gathers the full [2048, 1] output.

v2 design (vs v1 baseline at 1.47ms HW):
  - hypergraph propagation with row-sharded H (fp8 0/1), bf16 activations:
    pass A streams H wide (x chunks stationary, 8 psum banks), then the
    [D,G] partial is PE-transposed + de^-1-scaled BEFORE the AllReduce so
    the reduced s_n arrives in (g,d) layout ready to be pass-B weights.
    pass B streams H^T panels (s_n chunks stationary), output y^T is
    PE-transposed back to x layout with dv^-1 folded into the scalar-
    engine copy.  Zero DMA transposes, zero DRAM activation roundtrips.
  - per-mat AllReduces overlap the other matrices' compute.
  - member-attention tail: item projections precomputed during prop;
    packed [user | user@W1u] table AllGathered; ragged dma_gather split
    across 4 DMA queues; segment softmax via host-built one-hot matmuls.
"""
import sys
sys.path.insert(0, '/opt/trn_rl_repo')

import numpy as np
import ml_dtypes

import concourse.bass as bass
import concourse.mybir as mybir
import concourse.tile as tile
from concourse import bacc
from concourse.bass_utils import run_bass_kernel_spmd
from concourse.masks import make_identity

bf16 = ml_dtypes.bfloat16
f8 = ml_dtypes.float8_e4m3fn
FP32 = mybir.dt.float32
BF16 = mybir.dt.bfloat16
F8 = mybir.dt.float8e4
I16 = mybir.dt.int16

NC = 8
U, G, D, B = 30000, 4096, 128, 2048
UC = U // NC            # 3750 local users
KU = 30                 # user chunks of 128 (padded)
UCP = KU * 128          # 3840
GS = 8                  # g-tiles of 512 in pass A
NGC = 32                # g chunks of 128
USUB = 480              # pass-B u-subtile width (8 * 480 = 3840)
NUS = 8
GGR = G // NC           # 512 local H_gg rows
KG = 4                  # gg row chunks of 128
BC = B // NC            # 256 batch rows per core

AF = mybir.ActivationFunctionType


def _wrap_idx(idx, n):
    cols = (n + 15) // 16
    w = np.zeros((16, cols), np.int16)
    for i in range(n):
        w[i % 16, i // 16] = idx[i]
    return np.tile(w, (8, 1))


def _prep(inputs):
    inp = {k: np.asarray(v) for k, v in inputs.items()}
    H = {'a': inp['H_ug'].astype(np.float32),
         'b': inp['H_ug_affect'].astype(np.float32)}
    Hg = inp['H_gg'].astype(np.float32)
    user_emb = inp['user_emb'].astype(np.float32)
    group_emb = inp['group_emb'].astype(np.float32)
    item_emb = inp['item_emb'].astype(np.float32)
    groupid = inp['groupid'].astype(np.int64)
    itemid = inp['itemid'].astype(np.int64)
    mids = inp['member_user_ids'].astype(np.int64)
    bseg = inp['batch_seg'].astype(np.int64)

    att_w1 = inp['att_w1'].astype(np.float32)
    att_b1 = inp['att_b1'].astype(np.float32)
    att_w2 = inp['att_w2'].astype(np.float32)
    pw1 = inp['pred_w1'].astype(np.float32)
    pb1 = inp['pred_b1'].astype(np.float32)
    pw2 = inp['pred_w2'].astype(np.float32)

    deg = {}
    for m, Hm in (('a', H['a']), ('b', H['b']), ('g', Hg)):
        deg[m] = (Hm.sum(1) + 1e-5, Hm.sum(0) + 1e-5)

    counts = np.bincount(bseg, minlength=B)
    starts = np.concatenate([[0], np.cumsum(counts)])
    mc = [int(starts[(c + 1) * BC] - starts[c * BC]) for c in range(NC)]
    MPAD = int(-(-max(mc) // 128) * 128)
    NJ = MPAD // 128

    item_b = item_emb[itemid]                      # [B, D] host gather of inputs

    in_maps = []
    for c in range(NC):
        m = {}
        for k in ('a', 'b'):
            rows = slice(c * UC, (c + 1) * UC)
            Hp = np.zeros((UCP, G), np.float32)
            Hp[:UC] = H[k][rows]
            # hu: [128, KU, G]  (partition = user-within-chunk)
            m[f'hu_{k}'] = np.ascontiguousarray(
                Hp.reshape(KU, 128, G).transpose(1, 0, 2)).astype(f8)
            # hut: [NUS, 128, NGC*USUB]  (partition = g-within-chunk)
            HT = Hp.T.reshape(NGC, 128, NUS, USUB).transpose(2, 1, 0, 3)
            m[f'hut_{k}'] = np.ascontiguousarray(
                HT.reshape(NUS, 128, NGC * USUB)).astype(f8)
            dv, de = deg[k]
            dvp = np.ones((UCP,), np.float32)
            dvp[:UC] = 1.0 / dv[rows]
            # per-partition 1/dv for it0 x-refresh: [128, KU]
            m[f'dvr_{k}'] = np.ascontiguousarray(
                dvp.reshape(KU, 128).T).astype(np.float32)
            # (d,u)-layout 0.5/dv for the final combine: [128, UCP] bf16
            dvh = np.zeros((UCP,), np.float32)
            dvh[:UC] = 0.5 / dv[rows]
            m[f'dvsl_{k}'] = np.tile(dvh[None, :], (128, 1)).astype(bf16)
            # per-partition 1/de: [128, NGC]
            m[f'der_{k}'] = np.ascontiguousarray(
                (1.0 / de).reshape(NGC, 128).T).astype(np.float32)
        x0 = np.zeros((UCP, D), np.float32)
        x0[:UC] = user_emb[c * UC:(c + 1) * UC]
        m['x0u'] = np.ascontiguousarray(
            x0.reshape(KU, 128, D).transpose(1, 0, 2)).astype(bf16)

        rows = slice(c * GGR, (c + 1) * GGR)
        Hgl = Hg[rows]
        m['hg'] = np.ascontiguousarray(
            Hgl.reshape(KG, 128, G).transpose(1, 0, 2)).astype(f8)
        m['hgt'] = np.ascontiguousarray(
            Hgl.T.reshape(NGC, 128, GGR).transpose(1, 0, 2)).astype(f8)
        dv, de = deg['g']
        m['dvr_g'] = np.ascontiguousarray(
            (1.0 / dv[rows]).reshape(KG, 128).T).astype(np.float32)
        m['der_g'] = np.ascontiguousarray(
            (1.0 / de).reshape(NGC, 128).T).astype(np.float32)
        m['xg0'] = np.ascontiguousarray(
            group_emb[rows].reshape(KG, 128, D).transpose(1, 0, 2)).astype(bf16)

        bid = slice(c * BC, (c + 1) * BC)
        gid = groupid[bid]
        Hgr = Hg[gid] / deg['g'][0][gid][:, None]          # [BC, G]
        HgrT = Hgr.T.reshape(NGC, 128, 2, 128).transpose(1, 0, 2, 3)
        m['hgrt'] = np.ascontiguousarray(
            HgrT.reshape(128, NGC * 2 * 128)).astype(bf16)

        m['item_bt'] = np.ascontiguousarray(item_b[bid].T).astype(bf16)
        mlo, mhi = int(starts[c * BC]), int(starts[(c + 1) * BC])
        mid_c = mids[mlo:mhi]
        seg_c = (bseg[mlo:mhi] - c * BC).astype(np.int64)
        Mc = len(mid_c)
        gi = (mid_c // UC) * UCP + (mid_c % UC)
        gi = np.concatenate([gi, np.zeros(MPAD - Mc, np.int64)])
        m['gidx'] = _wrap_idx(gi.astype(np.int16), MPAD)
        S_bm = np.zeros((NJ, BC, 128), np.float32)
        S_mb = np.zeros((NJ, 128, BC), np.float32)
        jj, pp = np.arange(Mc) // 128, np.arange(Mc) % 128
        S_bm[jj, seg_c, pp] = 1.0
        S_mb[jj, pp, seg_c] = 1.0
        sbm = S_bm.reshape(NJ, 2, 128, 128).transpose(2, 0, 1, 3)
        smb = S_mb.reshape(NJ, 128, 2, 128).transpose(1, 0, 2, 3)
        m['s_bm'] = np.ascontiguousarray(sbm.reshape(128, NJ * 2 * 128)).astype(bf16)
        m['s_mb'] = np.ascontiguousarray(smb.reshape(128, NJ * 2 * 128)).astype(bf16)

        m['w1u'] = att_w1[:D].astype(bf16)
        m['w1i'] = att_w1[D:].astype(bf16)
        m['pw1'] = np.ascontiguousarray(
            pw1.reshape(3, 128, 8).transpose(1, 0, 2).reshape(128, 24)).astype(bf16)
        crow = np.zeros((1, 48), np.float32)
        crow[0, 0:16] = att_b1
        crow[0, 16:32] = att_w2[:, 0]
        crow[0, 32:40] = pb1
        crow[0, 40:48] = pw2[:, 0]
        m['crow'] = np.tile(crow, (128, 1))
        in_maps.append(m)

    meta = dict(MPAD=MPAD, NJ=NJ,
                att_b2=float(inp['att_b2'][0]), pred_b2=float(inp['pred_b2'][0]))
    return in_maps, meta


def _build(meta):
    NJ, MPAD = meta['NJ'], meta['MPAD']
    att_b2, pred_b2 = meta['att_b2'], meta['pred_b2']

    nc = bacc.Bacc("TRN2", target_bir_lowering=False, num_swdge_queues=4)

    def din(name, shape, dt):
        return nc.dram_tensor(name, list(shape), dt, kind="ExternalInput")

    hu = {k: din(f'hu_{k}', (128, KU, G), F8) for k in 'ab'}
    hut = {k: din(f'hut_{k}', (NUS, 128, NGC * USUB), F8) for k in 'ab'}
    dvr = {k: din(f'dvr_{k}', (128, KU), FP32) for k in 'ab'}
    dvsl = {k: din(f'dvsl_{k}', (128, UCP), BF16) for k in 'ab'}
    der = {k: din(f'der_{k}', (128, NGC), FP32) for k in 'ab'}
    x0u = din('x0u', (128, KU, D), BF16)
    hg = din('hg', (128, KG, G), F8)
    hgt = din('hgt', (128, NGC, GGR), F8)
    dvr['g'] = din('dvr_g', (128, KG), FP32)
    der['g'] = din('der_g', (128, NGC), FP32)
    xg0 = din('xg0', (128, KG, D), BF16)
    hgrt = din('hgrt', (128, NGC * 2 * 128), BF16)
    item_bt = din('item_bt', (128, 2 * 128), BF16)
    gidx = din('gidx', (128, MPAD // 16), I16)
    s_bm = din('s_bm', (128, NJ * 2 * 128), BF16)
    s_mb = din('s_mb', (128, NJ * 2 * 128), BF16)
    w1u = din('w1u', (D, 16), BF16)
    w1i = din('w1i', (D, 16), BF16)
    pw1 = din('pw1', (128, 24), BF16)
    crow = din('crow', (128, 48), FP32)
    out = nc.dram_tensor('out', [BC, 1], FP32, kind="ExternalOutput")

    RG = [list(range(NC))]
    KCH = {'a': KU, 'b': KU, 'g': KG}

    with tile.TileContext(nc) as tc:
        with (
            tc.tile_pool(name="pers", bufs=1) as pers,
            tc.tile_pool(name="ps", bufs=1, space="PSUM") as ps,
            tc.tile_pool(name="dram", bufs=1, space="DRAM") as dr,
        ):
            # ---------------- persistent small tiles ----------------
            w1u_sb = pers.tile([D, 16], BF16, name="w1u_sb")
            nc.sync.dma_start(w1u_sb[:], w1u[:])
            w1i_sb = pers.tile([D, 16], BF16, name="w1i_sb")
            nc.sync.dma_start(w1i_sb[:], w1i[:])
            pw1_sb = pers.tile([128, 3, 8], BF16, name="pw1_sb")
            nc.sync.dma_start(pw1_sb[:], pw1[:].rearrange("p (k o) -> p k o", k=3))
            crow_sb = pers.tile([128, 48], FP32, name="crow_sb")
            nc.sync.dma_start(crow_sb[:], crow[:])
            crow16 = pers.tile([128, 48], BF16, name="crow16")
            nc.vector.tensor_copy(crow16[:], crow_sb[:])
            ibt_sb = pers.tile([128, 256], BF16, name="ibt_sb")
            nc.sync.dma_start(ibt_sb[:], item_bt[:])
            ident = pers.tile([128, 128], FP32, name="ident")
            make_identity(nc, ident[:])
            ident16 = pers.tile([128, 128], BF16, name="ident16")
            nc.vector.tensor_copy(ident16[:], ident[:])
            choose_sb = pers.tile([128, 2, 128], FP32, name="choose_sb")
            iproj = pers.tile([128, 2, 16], BF16, name="iproj")
            ip_all = pers.tile([128, NJ, 16], BF16, name="ip_all")
            sn_g_pers = pers.tile([128, NGC, D], BF16, name="sn_g_pers")

            # DRAM internals
            ar_in = {(k, it): dr.tile([128, G], BF16, name=f"arin_{k}{it}",
                                      tag=f"arin{k}{it}")
                     for k in 'abg' for it in range(2)}
            ar_out = {(k, it): dr.tile([128, G], BF16, name=f"arout_{k}{it}",
                                       tag=f"arout{k}{it}", addr_space="Shared")
                      for k in 'abg' for it in range(2)}
            table_loc = dr.tile([UCP, 256], BF16, name="table_loc")
            table_full = dr.tile([NC * UCP, 256], BF16, name="table_full",
                                 addr_space="Shared")

            # ================= propagation phase =================
            with (
                tc.tile_pool(name="hk_pool", bufs=2) as hkp,
                tc.tile_pool(name="panel_pool", bufs=2) as plp,
                tc.tile_pool(name="prop", bufs=1) as prop,
                tc.tile_pool(name="stg", bufs=1) as stg,
            ):
                # x tiles
                xa_t = prop.tile([128, KU, D], BF16, name="xa_sb")
                xg_t = prop.tile([128, KG, D], BF16, name="xg_sb")
                x_sb = {'0': xa_t, 'a': xa_t,
                        'b': prop.tile([128, KU, D], BF16, name="xb_sb"),
                        'g': xg_t, 'g1': xg_t}
                nc.sync.dma_start(x_sb['0'][:], x0u[:])
                nc.sync.dma_start(x_sb['g'][:], xg0[:])
                sn = {'a': prop.tile([128, NGC, D], BF16, name="sn_a"),
                      'b': prop.tile([128, NGC, D], BF16, name="sn_b"),
                      'g': sn_g_pers}
                der_sb = {}
                dvr_sb = {}
                for k in 'abg':
                    der_sb[k] = prop.tile([128, NGC], FP32, name=f"der_{k}_sb")
                    nc.sync.dma_start(der_sb[k][:], der[k][:])
                    kk = KU if k != 'g' else KG
                    dvr_sb[k] = prop.tile([128, kk], FP32, name=f"dvr_{k}_sb")
                    nc.sync.dma_start(dvr_sb[k][:], dvr[k][:])
                hg_sb = prop.tile([128, KG, G], F8, name="hg_sb")
                nc.sync.dma_start(hg_sb[:], hg[:])
                hgt_sb = prop.tile([128, NGC, GGR], F8, name="hgt_sb")
                nc.sync.dma_start(hgt_sb[:], hgt[:])
                yT = {'a': prop.tile([128, UCP], BF16, name="yTa"),
                      'b': prop.tile([128, UCP], BF16, name="yTb"),
                      'g': prop.tile([128, GGR], BF16, name="yTg")}

                def pass_a(m, it):
                    """s_loc = H^T x  ->  transpose -> *de^-1 -> AR."""
                    kch = KCH[m]
                    src = x_sb['0'] if (it == 0 and m in 'ab') else \
                        x_sb[m if not (m == 'g' and it == 1) else 'g1']
                    pst = [ps.tile([128, 512], FP32, name=f"pa{gt}",
                                   tag=f"pa{gt}") for gt in range(GS)]
                    if m == 'g':
                        for k in range(kch):
                            for gt in range(GS):
                                nc.tensor.matmul(
                                    pst[gt][:], lhsT=src[:, k, :],
                                    rhs=hg_sb[:, k, gt * 512:(gt + 1) * 512],
                                    start=(k == 0), stop=(k == kch - 1))
                    else:
                        for kp in range(kch // 2):
                            hk = hkp.tile([128, 2, G], F8, name="hk", tag="hk")
                            nc.sync.dma_start(
                                hk[:], hu[m][:, 2 * kp:2 * kp + 2, :])
                            for kk in range(2):
                                k = 2 * kp + kk
                                for gt in range(GS):
                                    nc.tensor.matmul(
                                        pst[gt][:], lhsT=src[:, k, :],
                                        rhs=hk[:, kk, gt * 512:(gt + 1) * 512],
                                        start=(k == 0), stop=(k == kch - 1))
                    sAT = stg.tile([128, G], BF16, name="sAT", tag="sAT")
                    for gt in range(GS):
                        nc.vector.tensor_copy(
                            sAT[:, gt * 512:(gt + 1) * 512], pst[gt][:])
                    sloc = stg.tile([128, NGC, 128], BF16, name="sloc",
                                    tag="sloc")
                    for gc in range(NGC):
                        pt = ps.tile([128, 128], BF16, name="ptr",
                                     tag=f"pa{gc % 2}")
                        nc.tensor.transpose(pt[:], sAT[:, gc * 128:(gc + 1) * 128],
                                            ident16[:])
                        nc.scalar.activation(sloc[:, gc, :], pt[:], AF.Copy,
                                             scale=der_sb[m][:, gc:gc + 1])
                    nc.sync.dma_start(
                        ar_in[(m, it)][:],
                        sloc[:].rearrange("p g d -> p (g d)"))
                    nc.gpsimd.collective_compute(
                        "AllReduce", mybir.AluOpType.add,
                        ins=[ar_in[(m, it)].opt()], outs=[ar_out[(m, it)].opt()],
                        replica_groups=RG)

                def load_sn(m, it):
                    nc.sync.dma_start(
                        sn[m][:].rearrange("p g d -> p (g d)"),
                        ar_out[(m, it)][:])

                def pass_b(m, it):
                    """y^T = s_n^T H^T ; it0: transpose back to x layout with
                    dv^-1; it1 (a/b): keep (d,u) layout scaled by 0.5/dv."""
                    if m == 'g':
                        pbg = ps.tile([128, GGR], FP32, name="pbg", tag="pa2")
                        for gc in range(NGC):
                            nc.tensor.matmul(
                                pbg[:], lhsT=sn['g'][:, gc, :],
                                rhs=hgt_sb[:, gc, :],
                                start=(gc == 0), stop=(gc == NGC - 1))
                        nc.vector.tensor_copy(yT['g'][:], pbg[:])
                        for k in range(KG):
                            pt = ps.tile([128, 128], BF16, name="ptx",
                                         tag=f"pa{4 + k % 2}")
                            nc.tensor.transpose(
                                pt[:], yT['g'][:, k * 128:(k + 1) * 128],
                                ident16[:])
                            nc.scalar.activation(
                                x_sb['g1'][:, k, :], pt[:], AF.Copy,
                                scale=dvr_sb['g'][:, k:k + 1])
                        return
                    for us in range(NUS):
                        panel = plp.tile([128, NGC * USUB], F8, name="panel",
                                         tag="panel")
                        nc.sync.dma_start(panel[:], hut[m][us])
                        pb = ps.tile([128, USUB], FP32, name="pb",
                                     tag=f"pa{2 + us % 2}")
                        for gc in range(NGC):
                            nc.tensor.matmul(
                                pb[:], lhsT=sn[m][:, gc, :],
                                rhs=panel[:, gc * USUB:(gc + 1) * USUB],
                                start=(gc == 0), stop=(gc == NGC - 1))
                        sl = slice(us * USUB, (us + 1) * USUB)
                        if it == 0:
                            nc.vector.tensor_copy(yT[m][:, sl], pb[:])
                        else:
                            dvs = dvsl_sb[m]
                            nc.vector.tensor_tensor(
                                out=yT[m][:, sl], in0=pb[:], in1=dvs[:, sl],
                                op=mybir.AluOpType.mult)
                    if it == 0:
                        for k in range(KU):
                            pt = ps.tile([128, 128], BF16, name="ptx",
                                         tag=f"pa{4 + k % 2}")
                            nc.tensor.transpose(
                                pt[:], yT[m][:, k * 128:(k + 1) * 128],
                                ident16[:])
                            nc.scalar.activation(
                                x_sb[m][:, k, :], pt[:], AF.Copy,
                                scale=dvr_sb[m][:, k:k + 1])

                # ---- item projections (independent of propagation) ----
                for h in range(2):
                    pi = ps.tile([128, 16], FP32, name="pi", tag="pa6")
                    nc.tensor.matmul(pi[:],
                                     lhsT=ibt_sb[:, h * 128:(h + 1) * 128],
                                     rhs=w1i_sb[:], start=True, stop=True)
                    nc.vector.tensor_copy(iproj[:, h, :], pi[:])
                nc.vector.tensor_tensor(
                    out=iproj[:], in0=iproj[:],
                    in1=crow16[:, 0:16].unsqueeze(1).to_broadcast([128, 2, 16]),
                    op=mybir.AluOpType.add)

                # ================= the 2-iteration propagation =================
                pass_a('a', 0)
                pass_a('b', 0)
                pass_a('g', 0)

                # member-item projection table (PE-idle window during AR a0)
                sbm_v = s_bm[:].rearrange("p (j h m) -> p j h m", j=NJ, h=2)
                with tc.tile_pool(name="sbmp", bufs=2) as sp:
                    for j0 in range(0, NJ, 8):
                        jn = min(8, NJ - j0)
                        sc = sp.tile([128, 8, 2, 128], BF16, name="sc",
                                     tag="sbmc")
                        nc.sync.dma_start(sc[:, :jn], sbm_v[:, j0:j0 + jn])
                        for j in range(jn):
                            pj = ps.tile([128, 16], FP32, name="pj", tag="pa7")
                            for h in range(2):
                                nc.tensor.matmul(pj[:], lhsT=sc[:, j, h, :],
                                                 rhs=iproj[:, h, :],
                                                 start=(h == 0), stop=(h == 1))
                            nc.vector.tensor_copy(ip_all[:, j0 + j, :], pj[:])

                dvsl_sb = {}
                for m in 'ab':
                    dvsl_sb[m] = prop.tile([128, UCP], BF16, name=f"dvsl_{m}_sb")
                    nc.sync.dma_start(dvsl_sb[m][:], dvsl[m][:])

                load_sn('a', 0)
                pass_b('a', 0)
                pass_a('a', 1)
                load_sn('b', 0)
                pass_b('b', 0)
                pass_a('b', 1)
                load_sn('g', 0)
                pass_b('g', 0)
                pass_a('g', 1)
                load_sn('a', 1)
                pass_b('a', 1)
                load_sn('b', 1)
                pass_b('b', 1)

                # user = 0.5*(ya/dva + yb/dvb)  (both already scaled)
                userT = yT['a']
                nc.vector.tensor_add(userT[:], yT['a'][:], yT['b'][:])

                # u-projection  upT = W1u^T user  [16, UCP]
                upT = prop.tile([16, UCP], BF16, name="upT")
                for us in range(NUS):
                    pu = ps.tile([16, USUB], FP32, name="pu", tag="pa6")
                    nc.tensor.matmul(pu[:], lhsT=w1u_sb[:],
                                     rhs=userT[:, us * USUB:(us + 1) * USUB],
                                     start=True, stop=True)
                    nc.vector.tensor_copy(upT[:, us * USUB:(us + 1) * USUB],
                                          pu[:])

                # build packed table rows [user(128) | uproj(16) | pad]
                tbl = prop.tile([128, KU, 256], BF16, name="tbl")
                nc.vector.memset(tbl[:], 0.0)
                for k in range(KU):
                    pt = ps.tile([128, 128], BF16, name="ptt",
                                 tag=f"pa{k % 2}")
                    nc.tensor.transpose(
                        pt[:], userT[:, k * 128:(k + 1) * 128], ident16[:])
                    nc.scalar.activation(tbl[:, k, 0:128], pt[:], AF.Copy)
                    pt2 = ps.tile([128, 16], BF16, name="ptt2",
                                  tag=f"pa{4 + k % 2}")
                    nc.tensor.transpose(
                        pt2[:], upT[:16, k * 128:(k + 1) * 128],
                        ident16[:16, :16])
                    nc.scalar.activation(tbl[:, k, 128:144], pt2[:], AF.Copy)
                nc.sync.dma_start(
                    table_loc[:].rearrange("(k p) e -> p k e", p=128), tbl[:])
                nc.gpsimd.collective_compute(
                    "AllGather", mybir.AluOpType.bypass,
                    ins=[table_loc.opt()], outs=[table_full.opt()],
                    replica_groups=RG)

                load_sn('g', 1)

            # ================= tail =================
            with tc.tile_pool(name="wtp", bufs=1) as wtp:
                wt = wtp.tile([128, NJ, 132], BF16, name="wt")
                att_bf = wtp.tile([128, NJ], BF16, name="att_bf")

                with tc.tile_pool(name="tailA", bufs=1) as ta:
                    idx_sb = ta.tile([128, MPAD // 16], I16, name="idx_sb")
                    nc.sync.dma_start(idx_sb[:], gidx[:])
                    gath = ta.tile([128, NJ, 256], BF16, name="gath")
                    NSP = 4
                    js = [NJ * i // NSP for i in range(NSP + 1)]
                    for i in range(NSP):
                        j0, j1 = js[i], js[i + 1]
                        if j1 <= j0:
                            continue
                        nc.gpsimd.dma_gather(
                            out_ap=gath[:, j0:j1, :], in_ap=table_full[:],
                            idxs_ap=idx_sb[:, j0 * 8:j1 * 8],
                            num_idxs=(j1 - j0) * 128,
                            num_idxs_reg=(j1 - j0) * 128,
                            elem_size=256, single_packet=False, queue_num=i)

                    # choose (PE work overlapping the AllGather + gather)
                    hgrt_sb = ta.tile([128, NGC, 2, 128], BF16, name="hgrt_sb")
                    nc.sync.dma_start(
                        hgrt_sb[:],
                        hgrt[:].rearrange("p (g h b) -> p g h b", g=NGC, h=2))
                    ps_ch = [ps.tile([128, 128], FP32, name=f"ch{h}",
                                     tag=f"pa{6 + h}") for h in range(2)]
                    for gc in range(NGC):
                        for h in range(2):
                            nc.tensor.matmul(
                                ps_ch[h][:], lhsT=hgrt_sb[:, gc, h, :],
                                rhs=sn_g_pers[:, gc, :],
                                start=(gc == 0), stop=(gc == NGC - 1))
                    for h in range(2):
                        nc.vector.tensor_copy(choose_sb[:, h, :], ps_ch[h][:])

                    h_all = ta.tile([128, NJ, 16], BF16, name="h_all")
                    nc.vector.tensor_add(h_all[:], gath[:, :, 128:144],
                                         ip_all[:])
                    nc.scalar.activation(h_all[:], h_all[:], AF.Relu)
                    hw = ta.tile([128, NJ, 16], FP32, name="hw")
                    nc.vector.tensor_tensor(
                        out=hw[:], in0=h_all[:],
                        in1=crow16[:, 16:32].unsqueeze(1)
                            .to_broadcast([128, NJ, 16]),
                        op=mybir.AluOpType.mult)
                    logit = ta.tile([128, NJ], FP32, name="logit")
                    nc.vector.reduce_sum(logit[:], hw[:], axis=mybir.AxisListType.X)
                    att = ta.tile([128, NJ], FP32, name="att")
                    nc.scalar.activation(att[:], logit[:], AF.Exp, bias=att_b2)
                    nc.vector.tensor_copy(att_bf[:], att[:])

                    nc.vector.tensor_tensor(
                        out=wt[:, :, 0:128], in0=gath[:, :, 0:128],
                        in1=att_bf[:].unsqueeze(2).to_broadcast([128, NJ, 128]),
                        op=mybir.AluOpType.mult)
                    nc.vector.tensor_copy(wt[:, :, 128:129], att_bf[:].unsqueeze(2))

                with tc.tile_pool(name="tailB", bufs=1) as tb:
                    smb_sb = tb.tile([128, NJ, 2, 128], BF16, name="smb_sb")
                    nc.sync.dma_start(
                        smb_sb[:],
                        s_mb[:].rearrange("p (j h b) -> p j h b", j=NJ, h=2))
                    ps_ag = [ps.tile([128, 129], FP32, name=f"ag{h}",
                                     tag=f"pa{2 + h}") for h in range(2)]
                    for j in range(NJ):
                        for h in range(2):
                            nc.tensor.matmul(ps_ag[h][:], lhsT=smb_sb[:, j, h, :],
                                             rhs=wt[:, j, 0:129],
                                             start=(j == 0), stop=(j == NJ - 1))

                    gT = tb.tile([128, 2, 128], BF16, name="gT")
                    for h in range(2):
                        den_r = tb.tile([128, 1], FP32, name="den_r", tag="den_r")
                        nc.vector.reciprocal(den_r[:], ps_ag[h][:, 128:129])
                        grp = tb.tile([128, 128], FP32, name="grp", tag="grp")
                        nc.vector.tensor_tensor(
                            out=grp[:], in0=ps_ag[h][:, 0:128],
                            in1=den_r[:].to_broadcast([128, 128]),
                            op=mybir.AluOpType.mult)
                        nc.vector.tensor_add(grp[:], grp[:], choose_sb[:, h, :])
                        pt = ps.tile([128, 128], FP32, name="pt", tag="pa4")
                        nc.tensor.transpose(pt[:], grp[:], ident[:])
                        nc.vector.tensor_copy(gT[:, h, :], pt[:])

                    giT = tb.tile([128, 2, 128], BF16, name="giT")
                    nc.vector.tensor_tensor(
                        out=giT[:], in0=gT[:],
                        in1=ibt_sb[:].rearrange("p (h b) -> p h b", h=2),
                        op=mybir.AluOpType.mult)

                    out_sb = tb.tile([128, 2], FP32, name="out_sb")
                    for h in range(2):
                        pp = ps.tile([128, 8], FP32, name="pp", tag="pa5")
                        ne = [giT[:, h, :], gT[:, h, :],
                              ibt_sb[:, h * 128:(h + 1) * 128]]
                        for kk in range(3):
                            nc.tensor.matmul(pp[:], lhsT=ne[kk],
                                             rhs=pw1_sb[:, kk, :],
                                             start=(kk == 0), stop=(kk == 2))
                        h2 = tb.tile([128, 8], FP32, name="h2", tag="h2")
                        nc.vector.tensor_tensor(
                            out=h2[:], in0=pp[:],
                            in1=crow_sb[:, 32:40],
                            op=mybir.AluOpType.add)
                        nc.scalar.activation(h2[:], h2[:], AF.Relu)
                        nc.vector.tensor_tensor(
                            out=h2[:], in0=h2[:],
                            in1=crow_sb[:, 40:48],
                            op=mybir.AluOpType.mult)
                        l2 = tb.tile([128, 1], FP32, name="l2", tag="l2")
                        nc.vector.reduce_sum(l2[:], h2[:],
                                             axis=mybir.AxisListType.X)
                        nc.scalar.activation(out_sb[:, h:h + 1], l2[:],
                                             AF.Sigmoid, bias=pred_b2)
                    nc.sync.dma_start(
                        out[:].rearrange("(h p) o -> p h o", p=128),
                        out_sb[:].unsqueeze(2))

    nc.finalize()
    return nc


def kernel(**inputs):
    in_maps, meta = _prep(inputs)
    nc = _build(meta)
    res = run_bass_kernel_spmd(nc, in_maps, list(range(NC)))
    outs = [res.results[c]['out'] for c in range(NC)]
    return np.concatenate(outs, axis=0).astype(np.float32)


# revision 11
# speedup vs baseline: 1.7516x; 1.1358x over previous
"""Trainium2 Bass kernel for nn_ModelName_86242943303934 (gnn_message_passing).

Self-contained: takes FULL inputs, shards across 8 NeuronCores internally,
runs one SPMD Bass/Tile program, gathers the full [2048, 1] output.

v2 design (vs v1 baseline at 1.47ms HW):
  - hypergraph propagation with row-sharded H (fp8 0/1), bf16 activations:
    pass A streams H wide (x chunks stationary, 8 psum banks), then the
    [D,G] partial is PE-transposed + de^-1-scaled BEFORE the AllReduce so
    the reduced s_n arrives in (g,d) layout ready to be pass-B weights.
    pass B streams H^T panels (s_n chunks stationary), output y^T is
    PE-transposed back to x layout with dv^-1 folded into the scalar-
    engine copy.  Zero DMA transposes, zero DRAM activation roundtrips.
  - per-mat AllReduces overlap the other matrices' compute.
  - member-attention tail: item projections precomputed during prop;
    packed [user | user@W1u] table AllGathered; ragged dma_gather split
    across 4 DMA queues; segment softmax via host-built one-hot matmuls.
"""
import sys
sys.path.insert(0, '/opt/trn_rl_repo')

import numpy as np
import ml_dtypes

import concourse.bass as bass
import concourse.mybir as mybir
import concourse.tile as tile
from concourse import bacc
from concourse.bass_utils import run_bass_kernel_spmd
from concourse.masks import make_identity

bf16 = ml_dtypes.bfloat16
f8 = ml_dtypes.float8_e4m3fn
FP32 = mybir.dt.float32
BF16 = mybir.dt.bfloat16
F8 = mybir.dt.float8e4
I16 = mybir.dt.int16

NC = 8
U, G, D, B = 30000, 4096, 128, 2048
UC = U // NC            # 3750 local users
KU = 30                 # user chunks of 128 (padded)
UCP = KU * 128          # 3840
GS = 8                  # g-tiles of 512 in pass A
NGC = 32                # g chunks of 128
USUB = 480              # pass-B u-subtile width (8 * 480 = 3840)
NUS = 8
GGR = G // NC           # 512 local H_gg rows
KG = 4                  # gg row chunks of 128
BC = B // NC            # 256 batch rows per core

AF = mybir.ActivationFunctionType
DBG = False
FP8A = True   # fp8 DoubleRow for pass A (a/b mats)
FP8B = True   # fp8 DoubleRow for pass B (a/b mats)
DR = mybir.MatmulPerfMode.DoubleRow


def _wrap_idx(idx, n):
    cols = (n + 15) // 16
    w = np.zeros((16, cols), np.int16)
    for i in range(n):
        w[i % 16, i // 16] = idx[i]
    return np.tile(w, (8, 1))


def _prep(inputs):
    inp = {k: np.asarray(v) for k, v in inputs.items()}
    H = {'a': inp['H_ug'].astype(np.float32),
         'b': inp['H_ug_affect'].astype(np.float32)}
    Hg = inp['H_gg'].astype(np.float32)
    user_emb = inp['user_emb'].astype(np.float32)
    group_emb = inp['group_emb'].astype(np.float32)
    item_emb = inp['item_emb'].astype(np.float32)
    groupid = inp['groupid'].astype(np.int64)
    itemid = inp['itemid'].astype(np.int64)
    mids = inp['member_user_ids'].astype(np.int64)
    bseg = inp['batch_seg'].astype(np.int64)

    att_w1 = inp['att_w1'].astype(np.float32)
    att_b1 = inp['att_b1'].astype(np.float32)
    att_w2 = inp['att_w2'].astype(np.float32)
    pw1 = inp['pred_w1'].astype(np.float32)
    pb1 = inp['pred_b1'].astype(np.float32)
    pw2 = inp['pred_w2'].astype(np.float32)

    deg = {}
    for m, Hm in (('a', H['a']), ('b', H['b']), ('g', Hg)):
        deg[m] = (Hm.sum(1) + 1e-5, Hm.sum(0) + 1e-5)

    counts = np.bincount(bseg, minlength=B)
    starts = np.concatenate([[0], np.cumsum(counts)])
    mc = [int(starts[(c + 1) * BC] - starts[c * BC]) for c in range(NC)]
    MPAD = int(-(-max(mc) // 128) * 128)
    NJ = MPAD // 128

    item_b = item_emb[itemid]                      # [B, D] host gather of inputs

    in_maps = []
    for c in range(NC):
        m = {}
        for k in ('a', 'b'):
            rows = slice(c * UC, (c + 1) * UC)
            Hp = np.zeros((UCP, G), np.float32)
            Hp[:UC] = H[k][rows]
            # hu: [128, KU, G]  (partition = user-within-chunk)
            m[f'hu_{k}'] = np.ascontiguousarray(
                Hp.reshape(KU, 128, G).transpose(1, 0, 2)).astype(f8)
            # hut: [NUS, 128, NGC*USUB]  (partition = g-within-chunk)
            HT = Hp.T.reshape(NGC, 128, NUS, USUB).transpose(2, 1, 0, 3)
            m[f'hut_{k}'] = np.ascontiguousarray(
                HT.reshape(NUS, 128, NGC * USUB)).astype(f8)
            dv, de = deg[k]
            dvp = np.ones((UCP,), np.float32)
            dvp[:UC] = 1.0 / dv[rows]
            # per-partition 1/dv for it0 x-refresh: [128, KU]
            m[f'dvr_{k}'] = np.ascontiguousarray(
                dvp.reshape(KU, 128).T).astype(np.float32)
            # (d,u)-layout 0.5/dv for the final combine: [128, UCP] bf16
            dvh = np.zeros((UCP,), np.float32)
            dvh[:UC] = 0.5 / dv[rows]
            m[f'dvsl_{k}'] = np.tile(dvh[None, :], (128, 1)).astype(bf16)
            # per-partition 1/de: [128, NGC]
            m[f'der_{k}'] = np.ascontiguousarray(
                (1.0 / de).reshape(NGC, 128).T).astype(np.float32)
        x0 = np.zeros((UCP, D), np.float32)
        x0[:UC] = user_emb[c * UC:(c + 1) * UC]
        m['x0u'] = np.ascontiguousarray(
            x0.reshape(KU, 128, D).transpose(1, 0, 2)).astype(
                f8 if FP8A else bf16)

        rows = slice(c * GGR, (c + 1) * GGR)
        Hgl = Hg[rows]
        m['hg'] = np.ascontiguousarray(
            Hgl.reshape(KG, 128, G).transpose(1, 0, 2)).astype(f8)
        m['hgt'] = np.ascontiguousarray(
            Hgl.T.reshape(NGC, 128, GGR).transpose(1, 0, 2)).astype(f8)
        dv, de = deg['g']
        m['dvr_g'] = np.ascontiguousarray(
            (1.0 / dv[rows]).reshape(KG, 128).T).astype(np.float32)
        m['der_g'] = np.ascontiguousarray(
            (1.0 / de).reshape(NGC, 128).T).astype(np.float32)
        m['xg0'] = np.ascontiguousarray(
            group_emb[rows].reshape(KG, 128, D).transpose(1, 0, 2)).astype(bf16)

        bid = slice(c * BC, (c + 1) * BC)
        gid = groupid[bid]
        Hgr = Hg[gid] / deg['g'][0][gid][:, None]          # [BC, G]
        HgrT = Hgr.T.reshape(NGC, 128, 2, 128).transpose(1, 0, 2, 3)
        m['hgrt'] = np.ascontiguousarray(
            HgrT.reshape(128, NGC * 2 * 128)).astype(bf16)

        m['item_bt'] = np.ascontiguousarray(item_b[bid].T).astype(bf16)
        mlo, mhi = int(starts[c * BC]), int(starts[(c + 1) * BC])
        mid_c = mids[mlo:mhi]
        seg_c = (bseg[mlo:mhi] - c * BC).astype(np.int64)
        Mc = len(mid_c)
        gi = (mid_c // UC) * UCP + (mid_c % UC)
        gi = np.concatenate([gi, np.zeros(MPAD - Mc, np.int64)])
        m['gidx'] = _wrap_idx(gi.astype(np.int16), MPAD)
        S_bm = np.zeros((NJ, BC, 128), np.float32)
        S_mb = np.zeros((NJ, 128, BC), np.float32)
        jj, pp = np.arange(Mc) // 128, np.arange(Mc) % 128
        S_bm[jj, seg_c, pp] = 1.0
        S_mb[jj, pp, seg_c] = 1.0
        sbm = S_bm.reshape(NJ, 2, 128, 128).transpose(2, 0, 1, 3)
        smb = S_mb.reshape(NJ, 128, 2, 128).transpose(1, 0, 2, 3)
        m['s_bm'] = np.ascontiguousarray(sbm.reshape(128, NJ * 2 * 128)).astype(f8)
        m['s_mb'] = np.ascontiguousarray(smb.reshape(128, NJ * 2 * 128)).astype(f8)

        m['w1u'] = (att_w1[:D] * 64.0).astype(bf16)
        m['w1i'] = (att_w1[D:] * 64.0).astype(bf16)
        m['pw1'] = np.ascontiguousarray(
            pw1.reshape(3, 128, 8).transpose(1, 0, 2).reshape(128, 24)).astype(bf16)
        crow = np.zeros((1, 48), np.float32)
        crow[0, 0:16] = att_b1 * 64.0
        crow[0, 16:32] = att_w2[:, 0] / 64.0
        crow[0, 32:40] = pb1
        crow[0, 40:48] = pw2[:, 0]
        m['crow'] = np.tile(crow, (128, 1))
        in_maps.append(m)

    meta = dict(MPAD=MPAD, NJ=NJ,
                att_b2=float(inp['att_b2'][0]), pred_b2=float(inp['pred_b2'][0]))
    return in_maps, meta


def _build(meta):
    NJ, MPAD = meta['NJ'], meta['MPAD']
    att_b2, pred_b2 = meta['att_b2'], meta['pred_b2']

    nc = bacc.Bacc("TRN2", target_bir_lowering=False, num_swdge_queues=4)

    def din(name, shape, dt):
        return nc.dram_tensor(name, list(shape), dt, kind="ExternalInput")

    hu = {k: din(f'hu_{k}', (128, KU, G), F8) for k in 'ab'}
    hut = {k: din(f'hut_{k}', (NUS, 128, NGC * USUB), F8) for k in 'ab'}
    dvr = {k: din(f'dvr_{k}', (128, KU), FP32) for k in 'ab'}
    dvsl = {k: din(f'dvsl_{k}', (128, UCP), BF16) for k in 'ab'}
    der = {k: din(f'der_{k}', (128, NGC), FP32) for k in 'ab'}
    x0u = din('x0u', (128, KU, D), F8 if FP8A else BF16)
    hg = din('hg', (128, KG, G), F8)
    hgt = din('hgt', (128, NGC, GGR), F8)
    dvr['g'] = din('dvr_g', (128, KG), FP32)
    der['g'] = din('der_g', (128, NGC), FP32)
    xg0 = din('xg0', (128, KG, D), BF16)
    hgrt = din('hgrt', (128, NGC * 2 * 128), BF16)
    item_bt = din('item_bt', (128, 2 * 128), BF16)
    gidx = din('gidx', (128, MPAD // 16), I16)
    s_bm = din('s_bm', (128, NJ * 2 * 128), F8)
    s_mb = din('s_mb', (128, NJ * 2 * 128), F8)
    w1u = din('w1u', (D, 16), BF16)
    w1i = din('w1i', (D, 16), BF16)
    pw1 = din('pw1', (128, 24), BF16)
    crow = din('crow', (128, 48), FP32)
    out = nc.dram_tensor('out', [BC, 1], FP32, kind="ExternalOutput")
    dbg = {}
    if DBG:
        dbg['sn_a'] = nc.dram_tensor('dbg_sn_a', [128, G], F8 if FP8B else BF16,
                                     kind="ExternalOutput")
        dbg['x1_a'] = nc.dram_tensor('dbg_x1_a', [128, KU * D],
                                     F8 if FP8A else BF16, kind="ExternalOutput")
        dbg['user'] = nc.dram_tensor('dbg_user', [128, UCP], BF16,
                                     kind="ExternalOutput")
        dbg['ip'] = nc.dram_tensor('dbg_ip', [128, NJ * 16], BF16,
                                   kind="ExternalOutput")
        dbg['gath'] = nc.dram_tensor('dbg_gath', [128, NJ * 128], BF16,
                                     kind="ExternalOutput")
        dbg['tbl'] = nc.dram_tensor('dbg_tbl', [128, KU * 256], F8,
                                    kind="ExternalOutput")
        dbg['choose'] = nc.dram_tensor('dbg_choose', [128, 2 * 128], FP32,
                                       kind="ExternalOutput")

    RG = [list(range(NC))]
    KCH = {'a': KU, 'b': KU, 'g': KG}

    with tile.TileContext(nc) as tc:
        with (
            tc.tile_pool(name="pers", bufs=1) as pers,
            tc.tile_pool(name="gat", bufs=1) as gp,
            tc.tile_pool(name="ps", bufs=1, space="PSUM") as ps,
            tc.tile_pool(name="dram", bufs=1, space="DRAM") as dr,
        ):
            # ---------------- persistent small tiles ----------------
            w1u_sb = pers.tile([D, 16], BF16, name="w1u_sb")
            nc.sync.dma_start(w1u_sb[:], w1u[:])
            w1i_sb = pers.tile([D, 16], BF16, name="w1i_sb")
            nc.sync.dma_start(w1i_sb[:], w1i[:])
            pw1_sb = pers.tile([128, 3, 8], BF16, name="pw1_sb")
            nc.sync.dma_start(pw1_sb[:], pw1[:].rearrange("p (k o) -> p k o", k=3))
            crow_sb = pers.tile([128, 48], FP32, name="crow_sb")
            nc.sync.dma_start(crow_sb[:], crow[:])
            crow16 = pers.tile([128, 48], BF16, name="crow16")
            nc.vector.tensor_copy(crow16[:], crow_sb[:])
            ibt_sb = pers.tile([128, 256], BF16, name="ibt_sb")
            nc.sync.dma_start(ibt_sb[:], item_bt[:])
            ident = pers.tile([128, 128], FP32, name="ident")
            make_identity(nc, ident[:])
            ident16 = pers.tile([128, 128], BF16, name="ident16")
            nc.vector.tensor_copy(ident16[:], ident[:])
            choose_sb = pers.tile([128, 2, 128], FP32, name="choose_sb")
            iproj = pers.tile([128, 2, 16], BF16, name="iproj")
            ip_all = pers.tile([128, NJ, 16], BF16, name="ip_all")
            sn_g_pers = pers.tile([128, NGC, D], BF16, name="sn_g_pers")

            # DRAM internals
            ar_in = {(k, it): dr.tile([128, G], BF16, name=f"arin_{k}{it}",
                                      tag=f"arin{k}{it}")
                     for k in 'abg' for it in range(2)}
            ar_out = {(k, it): dr.tile([128, G], BF16, name=f"arout_{k}{it}",
                                       tag=f"arout{k}{it}", addr_space="Shared")
                      for k in 'abg' for it in range(2)}
            table_loc = dr.tile([UCP, 256], F8, name="table_loc")
            table_full = dr.tile([NC * UCP, 256], F8, name="table_full",
                                 addr_space="Shared")

            # ================= propagation phase =================
            with (
                tc.tile_pool(name="hk_pool", bufs=2) as hkp,
                tc.tile_pool(name="panel_pool", bufs=2) as plp,
                tc.tile_pool(name="prop", bufs=1) as prop,
                tc.tile_pool(name="stg", bufs=1) as stg,
            ):
                # x tiles
                XDT = F8 if FP8A else BF16
                xa_t = prop.tile([128, KU, D], XDT, name="xa_sb")
                xg_t = prop.tile([128, KG, D], BF16, name="xg_sb")
                x_sb = {'0': xa_t, 'a': xa_t,
                        'b': prop.tile([128, KU, D], XDT, name="xb_sb"),
                        'g': xg_t, 'g1': xg_t}
                nc.sync.dma_start(x_sb['0'][:], x0u[:])
                nc.sync.dma_start(x_sb['g'][:], xg0[:])
                SDT = F8 if FP8B else BF16
                sn = {'a': prop.tile([128, NGC, D], SDT, name="sn_a"),
                      'b': prop.tile([128, NGC, D], SDT, name="sn_b"),
                      'g': sn_g_pers}
                der_sb = {}
                dvr_sb = {}
                for k in 'abg':
                    der_sb[k] = prop.tile([128, NGC], FP32, name=f"der_{k}_sb")
                    nc.sync.dma_start(der_sb[k][:], der[k][:])
                    kk = KU if k != 'g' else KG
                    dvr_sb[k] = prop.tile([128, kk], FP32, name=f"dvr_{k}_sb")
                    nc.sync.dma_start(dvr_sb[k][:], dvr[k][:])
                hg_sb = prop.tile([128, KG, G], F8, name="hg_sb")
                hgt_sb = prop.tile([128, NGC, GGR], F8, name="hgt_sb")
                yT = {'a': prop.tile([128, UCP], BF16, name="yTa"),
                      'b': prop.tile([128, UCP], BF16, name="yTb"),
                      'g': prop.tile([128, GGR], BF16, name="yTg")}

                def pass_a(m, it):
                    """s_loc = H^T x  ->  transpose -> *de^-1 -> AR."""
                    kch = KCH[m]
                    src = x_sb['0'] if (it == 0 and m in 'ab') else \
                        x_sb[m if not (m == 'g' and it == 1) else 'g1']
                    pst = [ps.tile([128, 512], FP32, name=f"pa{gt}",
                                   tag=f"pa{gt}") for gt in range(GS)]
                    if m == 'g':
                        for k in range(kch):
                            for gt in range(GS):
                                nc.tensor.matmul(
                                    pst[gt][:], lhsT=src[:, k, :],
                                    rhs=hg_sb[:, k, gt * 512:(gt + 1) * 512],
                                    start=(k == 0), stop=(k == kch - 1))
                    else:
                        for kp in range(kch // 2):
                            hk = hkp.tile([128, 2, G], F8, name="hk", tag="hk")
                            eng = nc.sync if kp % 2 == 0 else nc.scalar
                            eng.dma_start(
                                hk[:], hu[m][:, 2 * kp:2 * kp + 2, :])
                            if FP8A:
                                for gt in range(GS):
                                    nc.tensor.matmul(
                                        pst[gt][:],
                                        lhsT=src[:, 2 * kp:2 * kp + 2, :],
                                        rhs=hk[:, 0:2, gt * 512:(gt + 1) * 512],
                                        start=(kp == 0),
                                        stop=(kp == kch // 2 - 1),
                                        perf_mode=DR)
                            else:
                                for kk in range(2):
                                    k = 2 * kp + kk
                                    for gt in range(GS):
                                        nc.tensor.matmul(
                                            pst[gt][:], lhsT=src[:, k, :],
                                            rhs=hk[:, kk,
                                                   gt * 512:(gt + 1) * 512],
                                            start=(k == 0),
                                            stop=(k == kch - 1))
                    sAT = stg.tile([128, G], BF16, name="sAT", tag="sAT")
                    for gt in range(GS):
                        nc.vector.tensor_copy(
                            sAT[:, gt * 512:(gt + 1) * 512], pst[gt][:])
                    sloc = stg.tile([128, NGC, 128], BF16, name="sloc",
                                    tag="sloc")
                    for gc in range(NGC):
                        pt = ps.tile([128, 128], BF16, name="ptr",
                                     tag=f"pa{gc % 2}")
                        nc.tensor.transpose(pt[:], sAT[:, gc * 128:(gc + 1) * 128],
                                            ident16[:])
                        if gc % 2 == 0:
                            nc.scalar.activation(sloc[:, gc, :], pt[:], AF.Copy,
                                                 scale=der_sb[m][:, gc:gc + 1])
                        else:
                            nc.vector.tensor_tensor(
                                out=sloc[:, gc, :], in0=pt[:],
                                in1=der_sb[m][:, gc:gc + 1]
                                    .to_broadcast([128, 128]),
                                op=mybir.AluOpType.mult)
                    nc.sync.dma_start(
                        ar_in[(m, it)][:],
                        sloc[:].rearrange("p g d -> p (g d)"))
                    nc.gpsimd.collective_compute(
                        "AllReduce", mybir.AluOpType.add,
                        ins=[ar_in[(m, it)].opt()], outs=[ar_out[(m, it)].opt()],
                        replica_groups=RG)

                def load_sn(m, it):
                    if m != 'g' and FP8B:
                        snb = stg.tile([128, G], BF16, name="snbf", tag="snbf")
                        nc.scalar.dma_start(snb[:], ar_out[(m, it)][:])
                        nc.vector.tensor_copy(
                            sn[m][:].rearrange("p g d -> p (g d)"), snb[:])
                    else:
                        nc.scalar.dma_start(
                            sn[m][:].rearrange("p g d -> p (g d)"),
                            ar_out[(m, it)][:])

                def pass_b(m, it):
                    """y^T = s_n^T H^T ; it0: transpose back to x layout with
                    dv^-1; it1 (a/b): keep (d,u) layout scaled by 0.5/dv."""
                    if m == 'g':
                        pbg = ps.tile([128, GGR], FP32, name="pbg", tag="pa2")
                        for gc in range(NGC):
                            nc.tensor.matmul(
                                pbg[:], lhsT=sn['g'][:, gc, :],
                                rhs=hgt_sb[:, gc, :],
                                start=(gc == 0), stop=(gc == NGC - 1))
                        nc.vector.tensor_copy(yT['g'][:], pbg[:])
                        for k in range(KG):
                            pt = ps.tile([128, 128], BF16, name="ptx",
                                         tag=f"pa{4 + k % 2}")
                            nc.tensor.transpose(
                                pt[:], yT['g'][:, k * 128:(k + 1) * 128],
                                ident16[:])
                            nc.scalar.activation(
                                x_sb['g1'][:, k, :], pt[:], AF.Copy,
                                scale=dvr_sb['g'][:, k:k + 1])
                        return
                    for us in range(NUS):
                        panel = plp.tile([128, NGC * USUB], F8, name="panel",
                                         tag="panel")
                        eng = nc.sync if us % 2 == 0 else nc.scalar
                        eng.dma_start(panel[:], hut[m][us])
                        pb = ps.tile([128, USUB], FP32, name="pb",
                                     tag=f"pa{2 + us % 2}")
                        if FP8B:
                            pv = panel[:].rearrange("p (g u) -> p g u", g=NGC)
                            for gcp in range(NGC // 2):
                                nc.tensor.matmul(
                                    pb[:],
                                    lhsT=sn[m][:, 2 * gcp:2 * gcp + 2, :],
                                    rhs=pv[:, 2 * gcp:2 * gcp + 2, :],
                                    start=(gcp == 0),
                                    stop=(gcp == NGC // 2 - 1),
                                    perf_mode=DR)
                        else:
                            for gc in range(NGC):
                                nc.tensor.matmul(
                                    pb[:], lhsT=sn[m][:, gc, :],
                                    rhs=panel[:, gc * USUB:(gc + 1) * USUB],
                                    start=(gc == 0), stop=(gc == NGC - 1))
                        sl = slice(us * USUB, (us + 1) * USUB)
                        if it == 0:
                            nc.vector.tensor_copy(yT[m][:, sl], pb[:])
                        else:
                            dvs = dvsl_sb[m]
                            nc.vector.tensor_tensor(
                                out=yT[m][:, sl], in0=pb[:], in1=dvs[:, sl],
                                op=mybir.AluOpType.mult)
                            if m == 'b':
                                # userT slice = ya' + yb' ; project it
                                nc.vector.tensor_add(yT['a'][:, sl],
                                                     yT['a'][:, sl],
                                                     yT['b'][:, sl])
                                pu = ps.tile([16, USUB], FP32, name="pu",
                                             tag="pa6")
                                nc.tensor.matmul(pu[:], lhsT=w1u_sb[:],
                                                 rhs=yT['a'][:, sl],
                                                 start=True, stop=True)
                                nc.vector.tensor_copy(upT[:, sl], pu[:])
                    if it == 0:
                        for k in range(KU):
                            pt = ps.tile([128, 128], BF16, name="ptx",
                                         tag=f"pa{4 + k % 2}")
                            nc.tensor.transpose(
                                pt[:], yT[m][:, k * 128:(k + 1) * 128],
                                ident16[:])
                            if k % 2 == 0:
                                nc.scalar.activation(
                                    x_sb[m][:, k, :], pt[:], AF.Copy,
                                    scale=dvr_sb[m][:, k:k + 1])
                            else:
                                nc.vector.tensor_tensor(
                                    out=x_sb[m][:, k, :], in0=pt[:],
                                    in1=dvr_sb[m][:, k:k + 1]
                                        .to_broadcast([128, 128]),
                                    op=mybir.AluOpType.mult)

                # ---- item projections (independent of propagation) ----
                for h in range(2):
                    pi = ps.tile([128, 16], FP32, name="pi", tag="pa6")
                    nc.tensor.matmul(pi[:],
                                     lhsT=ibt_sb[:, h * 128:(h + 1) * 128],
                                     rhs=w1i_sb[:], start=True, stop=True)
                    nc.vector.tensor_copy(iproj[:, h, :], pi[:])
                nc.vector.tensor_tensor(
                    out=iproj[:], in0=iproj[:],
                    in1=crow16[:, 0:16].unsqueeze(1).to_broadcast([128, 2, 16]),
                    op=mybir.AluOpType.add)

                # ================= the 2-iteration propagation =================
                pass_a('a', 0)
                nc.sync.dma_start(hg_sb[:], hg[:])
                nc.sync.dma_start(hgt_sb[:], hgt[:])
                pass_a('b', 0)
                pass_a('g', 0)

                # prepared ragged gather: descriptors generated now, DMA
                # triggered after the AllGather lands
                idx_sb = gp.tile([128, MPAD // 16], I16, name="idx_sb")
                nc.sync.dma_start(idx_sb[:], gidx[:])
                NSP = 4
                js = [NJ * i // NSP for i in range(NSP + 1)]
                gath_t = [gp.tile([128, js[i + 1] - js[i], 128], BF16,
                                  name=f"gath{i}") for i in range(NSP)]

                # member-item projection table (PE-idle window during AR a0)
                sbm_v = s_bm[:].rearrange("p (j h m) -> p j h m", j=NJ, h=2)
                with tc.tile_pool(name="sbmp", bufs=2) as sp:
                    for j0 in range(0, NJ, 8):
                        jn = min(8, NJ - j0)
                        sc = sp.tile([128, 8, 2, 128], F8, name="sc",
                                     tag="sbmc")
                        nc.sync.dma_start(sc[:, :jn], sbm_v[:, j0:j0 + jn])
                        for j in range(jn):
                            pj = ps.tile([128, 16], FP32, name="pj", tag="pa7")
                            for h in range(2):
                                nc.tensor.matmul(pj[:], lhsT=sc[:, j, h, :],
                                                 rhs=iproj[:, h, :],
                                                 start=(h == 0), stop=(h == 1))
                            nc.vector.tensor_copy(ip_all[:, j0 + j, :], pj[:])

                dvsl_sb = {}
                for m in 'ab':
                    dvsl_sb[m] = prop.tile([128, UCP], BF16, name=f"dvsl_{m}_sb")
                    nc.sync.dma_start(dvsl_sb[m][:], dvsl[m][:])

                load_sn('a', 0)
                if DBG:
                    nc.sync.dma_start(dbg['sn_a'][:],
                                      sn['a'][:].rearrange("p g d -> p (g d)"))
                pass_b('a', 0)
                if DBG:
                    nc.sync.dma_start(
                        dbg['x1_a'][:],
                        x_sb['a'][:].rearrange("p k d -> p (k d)"))
                pass_a('a', 1)
                load_sn('b', 0)
                pass_b('b', 0)
                pass_a('b', 1)
                load_sn('g', 0)
                pass_b('g', 0)
                pass_a('g', 1)
                upT = prop.tile([16, UCP], BF16, name="upT")
                load_sn('a', 1)
                pass_b('a', 1)
                load_sn('b', 1)
                pass_b('b', 1)

                userT = yT['a']

                # build packed table rows [user(128) | uproj(16) | pad]
                tbl = prop.tile([128, KU, 256], F8, name="tbl")
                nc.vector.memset(tbl[:], 0.0)
                for k in range(KU):
                    pt = ps.tile([128, 128], BF16, name="ptt",
                                 tag=f"pa{k % 2}")
                    nc.tensor.transpose(
                        pt[:], userT[:, k * 128:(k + 1) * 128], ident16[:])
                    nc.scalar.activation(tbl[:, k, 0:128], pt[:], AF.Copy,
                                         scale=64.0)
                    pt2 = ps.tile([128, 16], BF16, name="ptt2",
                                  tag=f"pa{4 + k % 2}")
                    nc.tensor.transpose(
                        pt2[:], upT[:16, k * 128:(k + 1) * 128],
                        ident16[:16, :16])
                    nc.scalar.activation(tbl[:, k, 128:144], pt2[:], AF.Copy)
                nc.sync.dma_start(
                    table_loc[:].rearrange("(k p) e -> p k e", p=128), tbl[:])
                if DBG:
                    nc.sync.dma_start(dbg['user'][:], userT[:])
                    nc.sync.dma_start(dbg['ip'][:],
                                      ip_all[:].rearrange("p j o -> p (j o)"))
                    nc.sync.dma_start(dbg['tbl'][:],
                                      tbl[:].rearrange("p k e -> p (k e)"))
                nc.gpsimd.collective_compute(
                    "AllGather", mybir.AluOpType.bypass,
                    ins=[table_loc.opt()], outs=[table_full.opt()],
                    replica_groups=RG)
                for i in range(NSP):
                    j0, j1 = js[i], js[i + 1]
                    nc.gpsimd.dma_gather(
                        out_ap=gath_t[i][:], in_ap=table_full[:].bitcast(BF16),
                        idxs_ap=idx_sb[:, j0 * 8:j1 * 8],
                        num_idxs=(j1 - j0) * 128,
                        num_idxs_reg=(j1 - j0) * 128,
                        elem_size=128, single_packet=False, queue_num=i)

                load_sn('g', 1)

            # ================= tail =================
            with tc.tile_pool(name="wtp", bufs=1) as wtp:
                wt_t = [wtp.tile([128, js[i + 1] - js[i], 132], BF16,
                                 name=f"wt{i}") for i in range(NSP)]
                smb_sb = wtp.tile([128, NJ, 2, 128], F8, name="smb_sb")
                nc.sync.dma_start(
                    smb_sb[:],
                    s_mb[:].rearrange("p (j h b) -> p j h b", j=NJ, h=2))

                with tc.tile_pool(name="tailA", bufs=1) as ta:
                    # choose (PE work overlapping the AllGather + gather)
                    hgrt_sb = ta.tile([128, NGC, 2, 128], BF16, name="hgrt_sb")
                    nc.sync.dma_start(
                        hgrt_sb[:],
                        hgrt[:].rearrange("p (g h b) -> p g h b", g=NGC, h=2))
                    ps_ch = [ps.tile([128, 128], FP32, name=f"ch{h}",
                                     tag=f"pa{6 + h}") for h in range(2)]
                    for gc in range(NGC):
                        for h in range(2):
                            nc.tensor.matmul(
                                ps_ch[h][:], lhsT=hgrt_sb[:, gc, h, :],
                                rhs=sn_g_pers[:, gc, :],
                                start=(gc == 0), stop=(gc == NGC - 1))
                    for h in range(2):
                        nc.vector.tensor_copy(choose_sb[:, h, :], ps_ch[h][:])

                    for i in range(NSP):
                        j0, j1 = js[i], js[i + 1]
                        nj = j1 - j0
                        gt = gath_t[i][:].bitcast(F8)
                        h_i = ta.tile([128, nj, 16], BF16, name=f"h{i}")
                        nc.vector.tensor_add(h_i[:], gt[:, :, 128:144],
                                             ip_all[:, j0:j1, :])
                        nc.scalar.activation(h_i[:], h_i[:], AF.Relu)
                        hw_i = ta.tile([128, nj, 16], FP32, name=f"hw{i}")
                        nc.vector.tensor_tensor(
                            out=hw_i[:], in0=h_i[:],
                            in1=crow16[:, 16:32].unsqueeze(1)
                                .to_broadcast([128, nj, 16]),
                            op=mybir.AluOpType.mult)
                        logit_i = ta.tile([128, nj], FP32, name=f"lg{i}")
                        nc.vector.reduce_sum(logit_i[:], hw_i[:],
                                             axis=mybir.AxisListType.X)
                        att_i = ta.tile([128, nj], FP32, name=f"at{i}")
                        nc.scalar.activation(att_i[:], logit_i[:], AF.Exp,
                                             bias=att_b2)
                        ab_i = ta.tile([128, nj], BF16, name=f"ab{i}")
                        nc.vector.tensor_copy(ab_i[:], att_i[:])
                        nc.vector.tensor_tensor(
                            out=wt_t[i][:, :, 0:128], in0=gt[:, :, 0:128],
                            in1=ab_i[:].unsqueeze(2)
                                .to_broadcast([128, nj, 128]),
                            op=mybir.AluOpType.mult)
                        nc.vector.tensor_copy(wt_t[i][:, :, 128:129],
                                              ab_i[:].unsqueeze(2))

                with tc.tile_pool(name="tailB", bufs=1) as tb:
                    ps_ag = [ps.tile([128, 129], FP32, name=f"ag{h}",
                                     tag=f"pa{2 + h}") for h in range(2)]
                    for i in range(NSP):
                        j0, j1 = js[i], js[i + 1]
                        for jl in range(j1 - j0):
                            j = j0 + jl
                            for h in range(2):
                                nc.tensor.matmul(
                                    ps_ag[h][:], lhsT=smb_sb[:, j, h, :],
                                    rhs=wt_t[i][:, jl, 0:129],
                                    start=(j == 0), stop=(j == NJ - 1))

                    gT = tb.tile([128, 2, 128], BF16, name="gT")
                    for h in range(2):
                        den64 = tb.tile([128, 1], FP32, name="den64",
                                        tag="den64")
                        nc.scalar.activation(den64[:], ps_ag[h][:, 128:129],
                                             AF.Copy, scale=64.0)
                        den_r = tb.tile([128, 1], FP32, name="den_r", tag="den_r")
                        nc.vector.reciprocal(den_r[:], den64[:])
                        grp = tb.tile([128, 128], FP32, name="grp", tag="grp")
                        nc.vector.tensor_tensor(
                            out=grp[:], in0=ps_ag[h][:, 0:128],
                            in1=den_r[:].to_broadcast([128, 128]),
                            op=mybir.AluOpType.mult)
                        nc.vector.tensor_add(grp[:], grp[:], choose_sb[:, h, :])
                        pt = ps.tile([128, 128], FP32, name="pt", tag="pa4")
                        nc.tensor.transpose(pt[:], grp[:], ident[:])
                        nc.vector.tensor_copy(gT[:, h, :], pt[:])

                    if DBG:
                        for i in range(NSP):
                            nc.sync.dma_start(
                                dbg['gath'][:, js[i] * 128:js[i + 1] * 128],
                                gath_t[i][:].rearrange("p j e -> p (j e)"))
                        nc.sync.dma_start(
                            dbg['choose'][:],
                            choose_sb[:].rearrange("p h b -> p (h b)"))
                    giT = tb.tile([128, 2, 128], BF16, name="giT")
                    nc.vector.tensor_tensor(
                        out=giT[:], in0=gT[:],
                        in1=ibt_sb[:].rearrange("p (h b) -> p h b", h=2),
                        op=mybir.AluOpType.mult)

                    out_sb = tb.tile([128, 2], FP32, name="out_sb")
                    for h in range(2):
                        pp = ps.tile([128, 8], FP32, name="pp", tag="pa5")
                        ne = [giT[:, h, :], gT[:, h, :],
                              ibt_sb[:, h * 128:(h + 1) * 128]]
                        for kk in range(3):
                            nc.tensor.matmul(pp[:], lhsT=ne[kk],
                                             rhs=pw1_sb[:, kk, :],
                                             start=(kk == 0), stop=(kk == 2))
                        h2 = tb.tile([128, 8], FP32, name="h2", tag="h2")
                        nc.vector.tensor_tensor(
                            out=h2[:], in0=pp[:],
                            in1=crow_sb[:, 32:40],
                            op=mybir.AluOpType.add)
                        nc.scalar.activation(h2[:], h2[:], AF.Relu)
                        nc.vector.tensor_tensor(
                            out=h2[:], in0=h2[:],
                            in1=crow_sb[:, 40:48],
                            op=mybir.AluOpType.mult)
                        l2 = tb.tile([128, 1], FP32, name="l2", tag="l2")
                        nc.vector.reduce_sum(l2[:], h2[:],
                                             axis=mybir.AxisListType.X)
                        nc.scalar.activation(out_sb[:, h:h + 1], l2[:],
                                             AF.Sigmoid, bias=pred_b2)
                    nc.sync.dma_start(
                        out[:].rearrange("(h p) o -> p h o", p=128),
                        out_sb[:].unsqueeze(2))

    nc.finalize()
    return nc


def kernel(**inputs):
    in_maps, meta = _prep(inputs)
    nc = _build(meta)
    res = run_bass_kernel_spmd(nc, in_maps, list(range(NC)))
    outs = [res.results[c]['out'] for c in range(NC)]
    return np.concatenate(outs, axis=0).astype(np.float32)


# revision 13
# speedup vs baseline: 1.8209x; 1.0395x over previous
"""Trainium2 Bass kernel for nn_ModelName_86242943303934 (gnn_message_passing).

Self-contained: takes FULL inputs, shards across 8 NeuronCores internally,
runs one SPMD Bass/Tile program, gathers the full [2048, 1] output.

v2 design (vs v1 baseline at 1.47ms HW):
  - hypergraph propagation with row-sharded H (fp8 0/1), bf16 activations:
    pass A streams H wide (x chunks stationary, 8 psum banks), then the
    [D,G] partial is PE-transposed + de^-1-scaled BEFORE the AllReduce so
    the reduced s_n arrives in (g,d) layout ready to be pass-B weights.
    pass B streams H^T panels (s_n chunks stationary), output y^T is
    PE-transposed back to x layout with dv^-1 folded into the scalar-
    engine copy.  Zero DMA transposes, zero DRAM activation roundtrips.
  - per-mat AllReduces overlap the other matrices' compute.
  - member-attention tail: item projections precomputed during prop;
    packed [user | user@W1u] table AllGathered; ragged dma_gather split
    across 4 DMA queues; segment softmax via host-built one-hot matmuls.
"""
import sys
sys.path.insert(0, '/opt/trn_rl_repo')

import numpy as np
import ml_dtypes

import concourse.bass as bass
import concourse.mybir as mybir
import concourse.tile as tile
from concourse import bacc
from concourse.bass_utils import run_bass_kernel_spmd
from concourse.masks import make_identity

bf16 = ml_dtypes.bfloat16
f8 = ml_dtypes.float8_e4m3fn
FP32 = mybir.dt.float32
BF16 = mybir.dt.bfloat16
F8 = mybir.dt.float8e4
I16 = mybir.dt.int16

NC = 8
U, G, D, B = 30000, 4096, 128, 2048
UC = U // NC            # 3750 local users
KU = 30                 # user chunks of 128 (padded)
UCP = KU * 128          # 3840
GS = 8                  # g-tiles of 512 in pass A
NGC = 32                # g chunks of 128
USUB = 480              # pass-B u-subtile width (8 * 480 = 3840)
NUS = 8
GGR = G // NC           # 512 local H_gg rows
KG = 4                  # gg row chunks of 128
BC = B // NC            # 256 batch rows per core

AF = mybir.ActivationFunctionType
DBG = False
FP8A = True   # fp8 DoubleRow for pass A (a/b mats)
FP8B = True   # fp8 DoubleRow for pass B (a/b mats)
DR = mybir.MatmulPerfMode.DoubleRow


def _wrap_idx(idx, n):
    cols = (n + 15) // 16
    w = np.zeros((16, cols), np.int16)
    for i in range(n):
        w[i % 16, i // 16] = idx[i]
    return np.tile(w, (8, 1))


def _prep(inputs):
    inp = {k: np.asarray(v) for k, v in inputs.items()}
    H = {'a': inp['H_ug'].astype(np.float32),
         'b': inp['H_ug_affect'].astype(np.float32)}
    Hg = inp['H_gg'].astype(np.float32)
    user_emb = inp['user_emb'].astype(np.float32)
    group_emb = inp['group_emb'].astype(np.float32)
    item_emb = inp['item_emb'].astype(np.float32)
    groupid = inp['groupid'].astype(np.int64)
    itemid = inp['itemid'].astype(np.int64)
    mids = inp['member_user_ids'].astype(np.int64)
    bseg = inp['batch_seg'].astype(np.int64)

    att_w1 = inp['att_w1'].astype(np.float32)
    att_b1 = inp['att_b1'].astype(np.float32)
    att_w2 = inp['att_w2'].astype(np.float32)
    pw1 = inp['pred_w1'].astype(np.float32)
    pb1 = inp['pred_b1'].astype(np.float32)
    pw2 = inp['pred_w2'].astype(np.float32)

    deg = {}
    for m, Hm in (('a', H['a']), ('b', H['b']), ('g', Hg)):
        deg[m] = (Hm.sum(1) + 1e-5, Hm.sum(0) + 1e-5)

    counts = np.bincount(bseg, minlength=B)
    starts = np.concatenate([[0], np.cumsum(counts)])
    mc = [int(starts[(c + 1) * BC] - starts[c * BC]) for c in range(NC)]
    MPAD = int(-(-max(mc) // 128) * 128)
    NJ = MPAD // 128

    item_b = item_emb[itemid]                      # [B, D] host gather of inputs

    in_maps = []
    for c in range(NC):
        m = {}
        for k in ('a', 'b'):
            rows = slice(c * UC, (c + 1) * UC)
            Hp = np.zeros((UCP, G), np.float32)
            Hp[:UC] = H[k][rows]
            # hu: [128, KU, G]  (partition = user-within-chunk)
            m[f'hu_{k}'] = np.ascontiguousarray(
                Hp.reshape(KU, 128, G).transpose(1, 0, 2)).astype(f8)
            # hut: [NUS, 128, NGC*USUB]  (partition = g-within-chunk)
            HT = Hp.T.reshape(NGC, 128, NUS, USUB).transpose(2, 1, 0, 3)
            m[f'hut_{k}'] = np.ascontiguousarray(
                HT.reshape(NUS, 128, NGC * USUB)).astype(f8)
            dv, de = deg[k]
            dvp = np.ones((UCP,), np.float32)
            dvp[:UC] = 1.0 / dv[rows]
            # per-partition 1/dv for it0 x-refresh: [128, KU]
            m[f'dvr_{k}'] = np.ascontiguousarray(
                dvp.reshape(KU, 128).T).astype(np.float32)
            # (d,u)-layout 0.5/dv for the final combine: [128, UCP] bf16
            dvh = np.zeros((UCP,), np.float32)
            dvh[:UC] = 0.5 / dv[rows]
            m[f'dvsl_{k}'] = np.tile(dvh[None, :], (128, 1)).astype(bf16)
            # per-partition 1/de: [128, NGC]
            m[f'der_{k}'] = np.ascontiguousarray(
                (1.0 / de).reshape(NGC, 128).T).astype(np.float32)
        x0 = np.zeros((UCP, D), np.float32)
        x0[:UC] = user_emb[c * UC:(c + 1) * UC]
        m['x0u'] = np.ascontiguousarray(
            x0.reshape(KU, 128, D).transpose(1, 0, 2)).astype(
                f8 if FP8A else bf16)

        rows = slice(c * GGR, (c + 1) * GGR)
        Hgl = Hg[rows]
        m['hg'] = np.ascontiguousarray(
            Hgl.reshape(KG, 128, G).transpose(1, 0, 2)).astype(f8)
        m['hgt'] = np.ascontiguousarray(
            Hgl.T.reshape(NGC, 128, GGR).transpose(1, 0, 2)).astype(f8)
        dv, de = deg['g']
        m['dvr_g'] = np.ascontiguousarray(
            (1.0 / dv[rows]).reshape(KG, 128).T).astype(np.float32)
        m['der_g'] = np.ascontiguousarray(
            (1.0 / de).reshape(NGC, 128).T).astype(np.float32)
        m['xg0'] = np.ascontiguousarray(
            group_emb[rows].reshape(KG, 128, D).transpose(1, 0, 2)).astype(bf16)

        bid = slice(c * BC, (c + 1) * BC)
        gid = groupid[bid]
        Hgr = Hg[gid] / deg['g'][0][gid][:, None]          # [BC, G]
        HgrT = Hgr.T.reshape(NGC, 128, 2, 128).transpose(1, 0, 2, 3)
        m['hgrt'] = np.ascontiguousarray(
            HgrT.reshape(128, NGC * 2 * 128)).astype(bf16)

        m['item_bt'] = np.ascontiguousarray(item_b[bid].T).astype(bf16)
        mlo, mhi = int(starts[c * BC]), int(starts[(c + 1) * BC])
        mid_c = mids[mlo:mhi]
        seg_c = (bseg[mlo:mhi] - c * BC).astype(np.int64)
        Mc = len(mid_c)
        gi = (mid_c // UC) * UCP + (mid_c % UC)
        gi = np.concatenate([gi, np.zeros(MPAD - Mc, np.int64)])
        m['gidx'] = _wrap_idx(gi.astype(np.int16), MPAD)
        S_bm = np.zeros((NJ, BC, 128), np.float32)
        S_mb = np.zeros((NJ, 128, BC), np.float32)
        jj, pp = np.arange(Mc) // 128, np.arange(Mc) % 128
        S_bm[jj, seg_c, pp] = 1.0
        S_mb[jj, pp, seg_c] = 1.0
        sbm = S_bm.reshape(NJ, 2, 128, 128).transpose(2, 0, 1, 3)
        smb = S_mb.reshape(NJ, 128, 2, 128).transpose(1, 0, 2, 3)
        m['s_bm'] = np.ascontiguousarray(sbm.reshape(128, NJ * 2 * 128)).astype(f8)
        m['s_mb'] = np.ascontiguousarray(smb.reshape(128, NJ * 2 * 128)).astype(f8)

        m['w1u'] = (att_w1[:D] * 64.0).astype(bf16)
        m['w1i'] = (att_w1[D:] * 64.0).astype(bf16)
        m['pw1'] = np.ascontiguousarray(
            pw1.reshape(3, 128, 8).transpose(1, 0, 2).reshape(128, 24)).astype(bf16)
        crow = np.zeros((1, 48), np.float32)
        crow[0, 0:16] = att_b1 * 64.0
        crow[0, 16:32] = att_w2[:, 0] / 64.0
        crow[0, 32:40] = pb1
        crow[0, 40:48] = pw2[:, 0]
        m['crow'] = np.tile(crow, (128, 1))
        in_maps.append(m)

    meta = dict(MPAD=MPAD, NJ=NJ,
                att_b2=float(inp['att_b2'][0]), pred_b2=float(inp['pred_b2'][0]))
    return in_maps, meta


def _build(meta):
    NJ, MPAD = meta['NJ'], meta['MPAD']
    att_b2, pred_b2 = meta['att_b2'], meta['pred_b2']

    nc = bacc.Bacc("TRN2", target_bir_lowering=False, num_swdge_queues=4)

    def din(name, shape, dt):
        return nc.dram_tensor(name, list(shape), dt, kind="ExternalInput")

    hu = {k: din(f'hu_{k}', (128, KU, G), F8) for k in 'ab'}
    hut = {k: din(f'hut_{k}', (NUS, 128, NGC * USUB), F8) for k in 'ab'}
    dvr = {k: din(f'dvr_{k}', (128, KU), FP32) for k in 'ab'}
    dvsl = {k: din(f'dvsl_{k}', (128, UCP), BF16) for k in 'ab'}
    der = {k: din(f'der_{k}', (128, NGC), FP32) for k in 'ab'}
    x0u = din('x0u', (128, KU, D), F8 if FP8A else BF16)
    hg = din('hg', (128, KG, G), F8)
    hgt = din('hgt', (128, NGC, GGR), F8)
    dvr['g'] = din('dvr_g', (128, KG), FP32)
    der['g'] = din('der_g', (128, NGC), FP32)
    xg0 = din('xg0', (128, KG, D), BF16)
    hgrt = din('hgrt', (128, NGC * 2 * 128), BF16)
    item_bt = din('item_bt', (128, 2 * 128), BF16)
    gidx = din('gidx', (128, MPAD // 16), I16)
    s_bm = din('s_bm', (128, NJ * 2 * 128), F8)
    s_mb = din('s_mb', (128, NJ * 2 * 128), F8)
    w1u = din('w1u', (D, 16), BF16)
    w1i = din('w1i', (D, 16), BF16)
    pw1 = din('pw1', (128, 24), BF16)
    crow = din('crow', (128, 48), FP32)
    out = nc.dram_tensor('out', [BC, 1], FP32, kind="ExternalOutput")
    dbg = {}
    if DBG:
        dbg['sn_a'] = nc.dram_tensor('dbg_sn_a', [128, G], F8 if FP8B else BF16,
                                     kind="ExternalOutput")
        dbg['x1_a'] = nc.dram_tensor('dbg_x1_a', [128, KU * D],
                                     F8 if FP8A else BF16, kind="ExternalOutput")
        dbg['user'] = nc.dram_tensor('dbg_user', [128, UCP], BF16,
                                     kind="ExternalOutput")
        dbg['ip'] = nc.dram_tensor('dbg_ip', [128, NJ * 16], BF16,
                                   kind="ExternalOutput")
        dbg['gath'] = nc.dram_tensor('dbg_gath', [128, NJ * 128], BF16,
                                     kind="ExternalOutput")
        dbg['tbl'] = nc.dram_tensor('dbg_tbl', [128, KU * 256], F8,
                                    kind="ExternalOutput")
        dbg['choose'] = nc.dram_tensor('dbg_choose', [128, 2 * 128], FP32,
                                       kind="ExternalOutput")

    RG = [list(range(NC))]
    KCH = {'a': KU, 'b': KU, 'g': KG}

    with tile.TileContext(nc) as tc:
        with (
            tc.tile_pool(name="pers", bufs=1) as pers,
            tc.tile_pool(name="gat", bufs=1) as gp,
            tc.tile_pool(name="ps", bufs=1, space="PSUM") as ps,
            tc.tile_pool(name="dram", bufs=1, space="DRAM") as dr,
        ):
            # ---------------- persistent small tiles ----------------
            w1u_sb = pers.tile([D, 16], BF16, name="w1u_sb")
            nc.sync.dma_start(w1u_sb[:], w1u[:])
            w1i_sb = pers.tile([D, 16], BF16, name="w1i_sb")
            nc.sync.dma_start(w1i_sb[:], w1i[:])
            pw1_sb = pers.tile([128, 3, 8], BF16, name="pw1_sb")
            nc.sync.dma_start(pw1_sb[:], pw1[:].rearrange("p (k o) -> p k o", k=3))
            crow_sb = pers.tile([128, 48], FP32, name="crow_sb")
            nc.sync.dma_start(crow_sb[:], crow[:])
            crow16 = pers.tile([128, 48], BF16, name="crow16")
            nc.vector.tensor_copy(crow16[:], crow_sb[:])
            ibt_sb = pers.tile([128, 256], BF16, name="ibt_sb")
            nc.sync.dma_start(ibt_sb[:], item_bt[:])
            ident = pers.tile([128, 128], FP32, name="ident")
            make_identity(nc, ident[:])
            ident16 = pers.tile([128, 128], BF16, name="ident16")
            nc.vector.tensor_copy(ident16[:], ident[:])
            choose_sb = pers.tile([128, 2, 128], FP32, name="choose_sb")
            iproj = pers.tile([128, 2, 16], BF16, name="iproj")
            ip_all = pers.tile([128, NJ, 16], BF16, name="ip_all")
            sn_g_pers = pers.tile([128, NGC, D], BF16, name="sn_g_pers")

            # DRAM internals
            ar_in = {(k, it): dr.tile([128, G], F8 if k != 'g' else BF16,
                                      name=f"arin_{k}{it}", tag=f"arin{k}{it}")
                     for k in 'abg' for it in range(2)}
            ar_out = {(k, it): dr.tile([128, G], F8 if k != 'g' else BF16,
                                       name=f"arout_{k}{it}",
                                       tag=f"arout{k}{it}", addr_space="Shared")
                      for k in 'abg' for it in range(2)}
            table_loc = dr.tile([UCP, 256], F8, name="table_loc")
            table_full = dr.tile([NC * UCP, 256], F8, name="table_full",
                                 addr_space="Shared")

            # ================= propagation phase =================
            with (
                tc.tile_pool(name="hk_pool", bufs=2) as hkp,
                tc.tile_pool(name="panel_pool", bufs=2) as plp,
                tc.tile_pool(name="prop", bufs=1) as prop,
                tc.tile_pool(name="stg", bufs=1) as stg,
            ):
                # x tiles
                XDT = F8 if FP8A else BF16
                xa_t = prop.tile([128, KU, D], XDT, name="xa_sb")
                xg_t = prop.tile([128, KG, D], BF16, name="xg_sb")
                x_sb = {'0': xa_t, 'a': xa_t,
                        'b': prop.tile([128, KU, D], XDT, name="xb_sb"),
                        'g': xg_t, 'g1': xg_t}
                nc.sync.dma_start(x_sb['0'][:], x0u[:])
                nc.sync.dma_start(x_sb['g'][:], xg0[:])
                SDT = F8 if FP8B else BF16
                sn = {'a': prop.tile([128, NGC, D], SDT, name="sn_a"),
                      'b': prop.tile([128, NGC, D], SDT, name="sn_b"),
                      'g': sn_g_pers}
                der_sb = {}
                dvr_sb = {}
                for k in 'abg':
                    der_sb[k] = prop.tile([128, NGC], FP32, name=f"der_{k}_sb")
                    nc.sync.dma_start(der_sb[k][:], der[k][:])
                    kk = KU if k != 'g' else KG
                    dvr_sb[k] = prop.tile([128, kk], FP32, name=f"dvr_{k}_sb")
                    nc.sync.dma_start(dvr_sb[k][:], dvr[k][:])
                hg_sb = prop.tile([128, KG, G], F8, name="hg_sb")
                hgt_sb = prop.tile([128, NGC, GGR], F8, name="hgt_sb")
                yT = {'a': prop.tile([128, UCP], BF16, name="yTa"),
                      'b': prop.tile([128, UCP], BF16, name="yTb"),
                      'g': prop.tile([128, GGR], BF16, name="yTg")}

                def pass_a(m, it):
                    """s_loc = H^T x  ->  transpose -> *de^-1 -> AR."""
                    kch = KCH[m]
                    src = x_sb['0'] if (it == 0 and m in 'ab') else \
                        x_sb[m if not (m == 'g' and it == 1) else 'g1']
                    pst = [ps.tile([128, 512], FP32, name=f"pa{gt}",
                                   tag=f"pa{gt}") for gt in range(GS)]
                    if m == 'g':
                        for k in range(kch):
                            for gt in range(GS):
                                nc.tensor.matmul(
                                    pst[gt][:], lhsT=src[:, k, :],
                                    rhs=hg_sb[:, k, gt * 512:(gt + 1) * 512],
                                    start=(k == 0), stop=(k == kch - 1))
                    else:
                        for kp in range(kch // 2):
                            hk = hkp.tile([128, 2, G], F8, name="hk", tag="hk")
                            eng = nc.sync if kp % 2 == 0 else nc.scalar
                            eng.dma_start(
                                hk[:], hu[m][:, 2 * kp:2 * kp + 2, :])
                            if FP8A:
                                for gt in range(GS):
                                    nc.tensor.matmul(
                                        pst[gt][:],
                                        lhsT=src[:, 2 * kp:2 * kp + 2, :],
                                        rhs=hk[:, 0:2, gt * 512:(gt + 1) * 512],
                                        start=(kp == 0),
                                        stop=(kp == kch // 2 - 1),
                                        perf_mode=DR)
                            else:
                                for kk in range(2):
                                    k = 2 * kp + kk
                                    for gt in range(GS):
                                        nc.tensor.matmul(
                                            pst[gt][:], lhsT=src[:, k, :],
                                            rhs=hk[:, kk,
                                                   gt * 512:(gt + 1) * 512],
                                            start=(k == 0),
                                            stop=(k == kch - 1))
                    sAT = stg.tile([128, G], BF16, name="sAT", tag="sAT")
                    for gt in range(GS):
                        nc.vector.tensor_copy(
                            sAT[:, gt * 512:(gt + 1) * 512], pst[gt][:])
                    sloc = stg.tile([128, NGC, 128],
                                    F8 if m != 'g' else BF16, name="sloc",
                                    tag="sloc")
                    for gc in range(NGC):
                        pt = ps.tile([128, 128], BF16, name="ptr",
                                     tag=f"pa{gc % 2}")
                        nc.tensor.transpose(pt[:], sAT[:, gc * 128:(gc + 1) * 128],
                                            ident16[:])
                        if gc % 2 == 0:
                            nc.scalar.activation(sloc[:, gc, :], pt[:], AF.Copy,
                                                 scale=der_sb[m][:, gc:gc + 1])
                        else:
                            nc.vector.tensor_tensor(
                                out=sloc[:, gc, :], in0=pt[:],
                                in1=der_sb[m][:, gc:gc + 1]
                                    .to_broadcast([128, 128]),
                                op=mybir.AluOpType.mult)
                    nc.sync.dma_start(
                        ar_in[(m, it)][:],
                        sloc[:].rearrange("p g d -> p (g d)"))
                    nc.gpsimd.collective_compute(
                        "AllReduce", mybir.AluOpType.add,
                        ins=[ar_in[(m, it)].opt()], outs=[ar_out[(m, it)].opt()],
                        replica_groups=RG)

                def load_sn(m, it):
                    nc.scalar.dma_start(
                        sn[m][:].rearrange("p g d -> p (g d)"),
                        ar_out[(m, it)][:])

                def pass_b(m, it):
                    """y^T = s_n^T H^T ; it0: transpose back to x layout with
                    dv^-1; it1 (a/b): keep (d,u) layout scaled by 0.5/dv."""
                    if m == 'g':
                        pbg = ps.tile([128, GGR], FP32, name="pbg", tag="pa2")
                        for gc in range(NGC):
                            nc.tensor.matmul(
                                pbg[:], lhsT=sn['g'][:, gc, :],
                                rhs=hgt_sb[:, gc, :],
                                start=(gc == 0), stop=(gc == NGC - 1))
                        nc.vector.tensor_copy(yT['g'][:], pbg[:])
                        for k in range(KG):
                            pt = ps.tile([128, 128], BF16, name="ptx",
                                         tag=f"pa{4 + k % 2}")
                            nc.tensor.transpose(
                                pt[:], yT['g'][:, k * 128:(k + 1) * 128],
                                ident16[:])
                            nc.scalar.activation(
                                x_sb['g1'][:, k, :], pt[:], AF.Copy,
                                scale=dvr_sb['g'][:, k:k + 1])
                        return
                    for us in range(NUS):
                        panel = plp.tile([128, NGC * USUB], F8, name="panel",
                                         tag="panel")
                        eng = nc.sync if us % 2 == 0 else nc.scalar
                        eng.dma_start(panel[:], hut[m][us])
                        pb = ps.tile([128, USUB], FP32, name="pb",
                                     tag=f"pa{2 + us % 2}")
                        if FP8B:
                            pv = panel[:].rearrange("p (g u) -> p g u", g=NGC)
                            for gcp in range(NGC // 2):
                                nc.tensor.matmul(
                                    pb[:],
                                    lhsT=sn[m][:, 2 * gcp:2 * gcp + 2, :],
                                    rhs=pv[:, 2 * gcp:2 * gcp + 2, :],
                                    start=(gcp == 0),
                                    stop=(gcp == NGC // 2 - 1),
                                    perf_mode=DR)
                        else:
                            for gc in range(NGC):
                                nc.tensor.matmul(
                                    pb[:], lhsT=sn[m][:, gc, :],
                                    rhs=panel[:, gc * USUB:(gc + 1) * USUB],
                                    start=(gc == 0), stop=(gc == NGC - 1))
                        sl = slice(us * USUB, (us + 1) * USUB)
                        if it == 0:
                            nc.vector.tensor_copy(yT[m][:, sl], pb[:])
                        else:
                            dvs = dvsl_sb[m]
                            nc.vector.tensor_tensor(
                                out=yT[m][:, sl], in0=pb[:], in1=dvs[:, sl],
                                op=mybir.AluOpType.mult)
                            if m == 'b':
                                # userT slice = ya' + yb' ; project it
                                nc.vector.tensor_add(yT['a'][:, sl],
                                                     yT['a'][:, sl],
                                                     yT['b'][:, sl])
                                pu = ps.tile([16, USUB], FP32, name="pu",
                                             tag="pa6")
                                nc.tensor.matmul(pu[:], lhsT=w1u_sb[:],
                                                 rhs=yT['a'][:, sl],
                                                 start=True, stop=True)
                                nc.vector.tensor_copy(upT[:, sl], pu[:])
                    if it == 0:
                        for k in range(KU):
                            pt = ps.tile([128, 128], BF16, name="ptx",
                                         tag=f"pa{4 + k % 2}")
                            nc.tensor.transpose(
                                pt[:], yT[m][:, k * 128:(k + 1) * 128],
                                ident16[:])
                            if k % 2 == 0:
                                nc.scalar.activation(
                                    x_sb[m][:, k, :], pt[:], AF.Copy,
                                    scale=dvr_sb[m][:, k:k + 1])
                            else:
                                nc.vector.tensor_tensor(
                                    out=x_sb[m][:, k, :], in0=pt[:],
                                    in1=dvr_sb[m][:, k:k + 1]
                                        .to_broadcast([128, 128]),
                                    op=mybir.AluOpType.mult)

                # ---- item projections (independent of propagation) ----
                for h in range(2):
                    pi = ps.tile([128, 16], FP32, name="pi", tag="pa6")
                    nc.tensor.matmul(pi[:],
                                     lhsT=ibt_sb[:, h * 128:(h + 1) * 128],
                                     rhs=w1i_sb[:], start=True, stop=True)
                    nc.vector.tensor_copy(iproj[:, h, :], pi[:])
                nc.vector.tensor_tensor(
                    out=iproj[:], in0=iproj[:],
                    in1=crow16[:, 0:16].unsqueeze(1).to_broadcast([128, 2, 16]),
                    op=mybir.AluOpType.add)

                # ================= the 2-iteration propagation =================
                nc.scalar.dma_start(hg_sb[:], hg[:])
                pass_a('g', 0)
                nc.scalar.dma_start(hgt_sb[:], hgt[:])
                pass_a('a', 0)
                pass_a('b', 0)

                # prepared ragged gather: descriptors generated now, DMA
                # triggered after the AllGather lands
                idx_sb = gp.tile([128, MPAD // 16], I16, name="idx_sb")
                nc.sync.dma_start(idx_sb[:], gidx[:])
                NSP = 4
                js = [NJ * i // NSP for i in range(NSP + 1)]
                gath_t = [gp.tile([128, js[i + 1] - js[i], 128], BF16,
                                  name=f"gath{i}") for i in range(NSP)]

                # member-item projection table (PE-idle window during AR a0)
                sbm_v = s_bm[:].rearrange("p (j h m) -> p j h m", j=NJ, h=2)
                with tc.tile_pool(name="sbmp", bufs=2) as sp:
                    for j0 in range(0, NJ, 8):
                        jn = min(8, NJ - j0)
                        sc = sp.tile([128, 8, 2, 128], F8, name="sc",
                                     tag="sbmc")
                        nc.sync.dma_start(sc[:, :jn], sbm_v[:, j0:j0 + jn])
                        for j in range(jn):
                            pj = ps.tile([128, 16], FP32, name="pj", tag="pa7")
                            for h in range(2):
                                nc.tensor.matmul(pj[:], lhsT=sc[:, j, h, :],
                                                 rhs=iproj[:, h, :],
                                                 start=(h == 0), stop=(h == 1))
                            nc.vector.tensor_copy(ip_all[:, j0 + j, :], pj[:])

                dvsl_sb = {}
                for m in 'ab':
                    dvsl_sb[m] = prop.tile([128, UCP], BF16, name=f"dvsl_{m}_sb")
                    nc.sync.dma_start(dvsl_sb[m][:], dvsl[m][:])

                load_sn('g', 0)
                pass_b('g', 0)
                pass_a('g', 1)
                load_sn('a', 0)
                if DBG:
                    nc.sync.dma_start(dbg['sn_a'][:],
                                      sn['a'][:].rearrange("p g d -> p (g d)"))
                pass_b('a', 0)
                if DBG:
                    nc.sync.dma_start(
                        dbg['x1_a'][:],
                        x_sb['a'][:].rearrange("p k d -> p (k d)"))
                pass_a('a', 1)
                load_sn('b', 0)
                pass_b('b', 0)
                pass_a('b', 1)
                # choose: consumes sn_g(it1); fills the AR_a1/AR_b1 windows
                load_sn('g', 1)
                hgrt_sb = prop.tile([128, NGC, 2, 128], BF16, name="hgrt_sb")
                nc.sync.dma_start(
                    hgrt_sb[:],
                    hgrt[:].rearrange("p (g h b) -> p g h b", g=NGC, h=2))
                ps_ch = [ps.tile([128, 128], FP32, name=f"ch{h}",
                                 tag=f"pa{6 + h}") for h in range(2)]
                for gc in range(NGC):
                    for h in range(2):
                        nc.tensor.matmul(
                            ps_ch[h][:], lhsT=hgrt_sb[:, gc, h, :],
                            rhs=sn_g_pers[:, gc, :],
                            start=(gc == 0), stop=(gc == NGC - 1))
                for h in range(2):
                    nc.vector.tensor_copy(choose_sb[:, h, :], ps_ch[h][:])
                upT = prop.tile([16, UCP], BF16, name="upT")
                load_sn('a', 1)
                pass_b('a', 1)
                load_sn('b', 1)
                pass_b('b', 1)

                userT = yT['a']

                # build packed table rows [user(128) | uproj(16) | pad]
                tbl = prop.tile([128, KU, 256], F8, name="tbl")
                nc.vector.memset(tbl[:], 0.0)
                for k in range(KU):
                    pt = ps.tile([128, 128], BF16, name="ptt",
                                 tag=f"pa{k % 2}")
                    nc.tensor.transpose(
                        pt[:], userT[:, k * 128:(k + 1) * 128], ident16[:])
                    nc.scalar.activation(tbl[:, k, 0:128], pt[:], AF.Copy,
                                         scale=64.0)
                    pt2 = ps.tile([128, 16], BF16, name="ptt2",
                                  tag=f"pa{4 + k % 2}")
                    nc.tensor.transpose(
                        pt2[:], upT[:16, k * 128:(k + 1) * 128],
                        ident16[:16, :16])
                    nc.scalar.activation(tbl[:, k, 128:144], pt2[:], AF.Copy)
                nc.sync.dma_start(
                    table_loc[:].rearrange("(k p) e -> p k e", p=128), tbl[:])
                if DBG:
                    nc.sync.dma_start(dbg['user'][:], userT[:])
                    nc.sync.dma_start(dbg['ip'][:],
                                      ip_all[:].rearrange("p j o -> p (j o)"))
                    nc.sync.dma_start(dbg['tbl'][:],
                                      tbl[:].rearrange("p k e -> p (k e)"))
                nc.gpsimd.collective_compute(
                    "AllGather", mybir.AluOpType.bypass,
                    ins=[table_loc.opt()], outs=[table_full.opt()],
                    replica_groups=RG)
                for i in range(NSP):
                    j0, j1 = js[i], js[i + 1]
                    nc.gpsimd.dma_gather(
                        out_ap=gath_t[i][:], in_ap=table_full[:].bitcast(BF16),
                        idxs_ap=idx_sb[:, j0 * 8:j1 * 8],
                        num_idxs=(j1 - j0) * 128,
                        num_idxs_reg=(j1 - j0) * 128,
                        elem_size=128, single_packet=False, queue_num=i)

            # ================= tail =================
            with tc.tile_pool(name="wtp", bufs=1) as wtp:
                wt_t = [wtp.tile([128, js[i + 1] - js[i], 132], BF16,
                                 name=f"wt{i}") for i in range(NSP)]
                smb_sb = wtp.tile([128, NJ, 2, 128], F8, name="smb_sb")
                nc.sync.dma_start(
                    smb_sb[:],
                    s_mb[:].rearrange("p (j h b) -> p j h b", j=NJ, h=2))

                with tc.tile_pool(name="tailA", bufs=1) as ta:
                    for i in range(NSP):
                        j0, j1 = js[i], js[i + 1]
                        nj = j1 - j0
                        gt = gath_t[i][:].bitcast(F8)
                        h_i = ta.tile([128, nj, 16], BF16, name=f"h{i}")
                        nc.vector.tensor_add(h_i[:], gt[:, :, 128:144],
                                             ip_all[:, j0:j1, :])
                        nc.scalar.activation(h_i[:], h_i[:], AF.Relu)
                        hw_i = ta.tile([128, nj, 16], FP32, name=f"hw{i}")
                        nc.vector.tensor_tensor(
                            out=hw_i[:], in0=h_i[:],
                            in1=crow16[:, 16:32].unsqueeze(1)
                                .to_broadcast([128, nj, 16]),
                            op=mybir.AluOpType.mult)
                        logit_i = ta.tile([128, nj], FP32, name=f"lg{i}")
                        nc.vector.reduce_sum(logit_i[:], hw_i[:],
                                             axis=mybir.AxisListType.X)
                        att_i = ta.tile([128, nj], FP32, name=f"at{i}")
                        nc.scalar.activation(att_i[:], logit_i[:], AF.Exp,
                                             bias=att_b2)
                        ab_i = ta.tile([128, nj], BF16, name=f"ab{i}")
                        nc.vector.tensor_copy(ab_i[:], att_i[:])
                        nc.vector.tensor_tensor(
                            out=wt_t[i][:, :, 0:128], in0=gt[:, :, 0:128],
                            in1=ab_i[:].unsqueeze(2)
                                .to_broadcast([128, nj, 128]),
                            op=mybir.AluOpType.mult)
                        nc.vector.tensor_copy(wt_t[i][:, :, 128:129],
                                              ab_i[:].unsqueeze(2))

                with tc.tile_pool(name="tailB", bufs=1) as tb:
                    ps_ag = [ps.tile([128, 129], FP32, name=f"ag{h}",
                                     tag=f"pa{2 + h}") for h in range(2)]
                    for i in range(NSP):
                        j0, j1 = js[i], js[i + 1]
                        for jl in range(j1 - j0):
                            j = j0 + jl
                            for h in range(2):
                                nc.tensor.matmul(
                                    ps_ag[h][:], lhsT=smb_sb[:, j, h, :],
                                    rhs=wt_t[i][:, jl, 0:129],
                                    start=(j == 0), stop=(j == NJ - 1))

                    gT = tb.tile([128, 2, 128], BF16, name="gT")
                    for h in range(2):
                        den64 = tb.tile([128, 1], FP32, name="den64",
                                        tag="den64")
                        nc.scalar.activation(den64[:], ps_ag[h][:, 128:129],
                                             AF.Copy, scale=64.0)
                        den_r = tb.tile([128, 1], FP32, name="den_r", tag="den_r")
                        nc.vector.reciprocal(den_r[:], den64[:])
                        grp = tb.tile([128, 128], FP32, name="grp", tag="grp")
                        nc.vector.tensor_tensor(
                            out=grp[:], in0=ps_ag[h][:, 0:128],
                            in1=den_r[:].to_broadcast([128, 128]),
                            op=mybir.AluOpType.mult)
                        nc.vector.tensor_add(grp[:], grp[:], choose_sb[:, h, :])
                        pt = ps.tile([128, 128], FP32, name="pt", tag="pa4")
                        nc.tensor.transpose(pt[:], grp[:], ident[:])
                        nc.vector.tensor_copy(gT[:, h, :], pt[:])

                    if DBG:
                        for i in range(NSP):
                            nc.sync.dma_start(
                                dbg['gath'][:, js[i] * 128:js[i + 1] * 128],
                                gath_t[i][:].rearrange("p j e -> p (j e)"))
                        nc.sync.dma_start(
                            dbg['choose'][:],
                            choose_sb[:].rearrange("p h b -> p (h b)"))
                    giT = tb.tile([128, 2, 128], BF16, name="giT")
                    nc.vector.tensor_tensor(
                        out=giT[:], in0=gT[:],
                        in1=ibt_sb[:].rearrange("p (h b) -> p h b", h=2),
                        op=mybir.AluOpType.mult)

                    out_sb = tb.tile([128, 2], FP32, name="out_sb")
                    for h in range(2):
                        pp = ps.tile([128, 8], FP32, name="pp", tag="pa5")
                        ne = [giT[:, h, :], gT[:, h, :],
                              ibt_sb[:, h * 128:(h + 1) * 128]]
                        for kk in range(3):
                            nc.tensor.matmul(pp[:], lhsT=ne[kk],
                                             rhs=pw1_sb[:, kk, :],
                                             start=(kk == 0), stop=(kk == 2))
                        h2 = tb.tile([128, 8], FP32, name="h2", tag="h2")
                        nc.vector.tensor_tensor(
                            out=h2[:], in0=pp[:],
                            in1=crow_sb[:, 32:40],
                            op=mybir.AluOpType.add)
                        nc.scalar.activation(h2[:], h2[:], AF.Relu)
                        nc.vector.tensor_tensor(
                            out=h2[:], in0=h2[:],
                            in1=crow_sb[:, 40:48],
                            op=mybir.AluOpType.mult)
                        l2 = tb.tile([128, 1], FP32, name="l2", tag="l2")
                        nc.vector.reduce_sum(l2[:], h2[:],
                                             axis=mybir.AxisListType.X)
                        nc.scalar.activation(out_sb[:, h:h + 1], l2[:],
                                             AF.Sigmoid, bias=pred_b2)
                    nc.sync.dma_start(
                        out[:].rearrange("(h p) o -> p h o", p=128),
                        out_sb[:].unsqueeze(2))

    nc.finalize()
    return nc


def kernel(**inputs):
    in_maps, meta = _prep(inputs)
    nc = _build(meta)
    res = run_bass_kernel_spmd(nc, in_maps, list(range(NC)))
    outs = [res.results[c]['out'] for c in range(NC)]
    return np.concatenate(outs, axis=0).astype(np.float32)
